# revision 4
# baseline (speedup 1.0000x reference)
"""nn_ARMonocularModel: KV-cache AR transformer on 8 TRN2 NeuronCores.

Reformulation: the reference recomputes the full causal encoder each of the
20 AR steps; under causal masking that equals prefill-once + KV-cache decode
(earlier token activations are step-invariant). 2.19 TFLOP -> ~0.11 TFLOP.

Device: ONE SPMD NEFF over 8 cores runs the full 3-layer prefill
(S=1041 tokens, B=2). Sharding = data-parallel over batch (cores 0-3 get
batch 0, cores 4-7 batch 1) x Megatron tensor-parallel over the 16 heads
(4/core) and the 3072 FFN cols (768/core) within each group, with bf16
AllReduce after out-proj and ff2 (replica groups [[0..3],[4..7]]).

Layout: activations live TRANSPOSED (xT [768, S]) so every GEMM is
weight-stationary (lhsT = W-tile) with xT as the moving operand, and
attention needs no transposes at all:
  scoresT[k,q] = kT-block.T @ qT   (per head; K-dim = head_dim pad 64)
  softmax along the PARTITION (key) axis via ones-matmul column sums
  (no max subtraction -- |scores| < 2 by construction, verified)
  ctxT[48,q]   = Vn-block.T @ expT (Vn = V in token-major layout)
LayerNorm stats (over D = partition axis) also use ones-matmul sums.
Biases are folded in as K=1 rank-1 matmuls (ones-row trick).

The 20-step serial decode (2 tokens/step) is latency-bound and runs on
host fp32 from the downloaded K/V caches; its FLOPs are negligible.
"""
import os
import numpy as np
import ml_dtypes

import concourse.bass as bass
import concourse.mybir as mybir
import concourse.tile as tile
from concourse import bacc
from concourse.bass_utils import run_bass_kernel_spmd

F32 = mybir.dt.float32
BF16 = mybir.dt.bfloat16
AF = mybir.ActivationFunctionType
ALU = mybir.AluOpType
NPBF16 = ml_dtypes.bfloat16

NC = 8
TP = 4                      # tensor-parallel group size
B, D, H, HD, L = 2, 768, 16, 48, 3
HDP = 64                    # head dim padded (zero cols) so 4 heads = 256 rows
NHC = H // TP               # heads per core = 4
N_IMG, N_PAST, T_DEC = 1024, 16, 20
S0 = N_IMG + 1 + N_PAST     # 1041
SP = 1152                   # padded tokens = 9 tiles of 128
NT = SP // 128              # 9
DK = D // 128               # 6
FFC = 4 * D // TP           # ff1 cols per core = 768
NEG = -30000.0

_cache = {}
import os as _os


def _build_prefill_neff():
    if "neff" in _cache:
        return _cache["neff"]
    nc = bacc.Bacc("TRN2", target_bir_lowering=False, debug=False, num_devices=NC)

    # ---- DRAM params (per-core) ----
    xt_in = nc.declare_dram_parameter("xt", [D, SP], BF16, isOutput=False)
    wqk = nc.declare_dram_parameter("wqk", [L, D, 2 * NHC * HDP], BF16, isOutput=False)
    wv = nc.declare_dram_parameter("wv", [L, D, NHC * HD], BF16, isOutput=False)
    wo = nc.declare_dram_parameter("wo", [L, NHC * HDP, D], BF16, isOutput=False)
    wf1 = nc.declare_dram_parameter("wf1", [L, D, FFC], BF16, isOutput=False)
    wf2 = nc.declare_dram_parameter("wf2", [L, FFC, D], BF16, isOutput=False)
    qkb = nc.declare_dram_parameter("qkb", [L, 2 * NHC * HDP, 1], F32, isOutput=False)
    vb = nc.declare_dram_parameter("vb", [L, 1, NHC * HD], BF16, isOutput=False)
    ob = nc.declare_dram_parameter("ob", [L, D, 1], F32, isOutput=False)
    f1b = nc.declare_dram_parameter("f1b", [L, FFC, 1], F32, isOutput=False)
    f2b = nc.declare_dram_parameter("f2b", [L, D, 1], F32, isOutput=False)
    g1 = nc.declare_dram_parameter("g1", [L, D, 1], F32, isOutput=False)
    b1 = nc.declare_dram_parameter("b1", [L, D, 1], F32, isOutput=False)
    g2 = nc.declare_dram_parameter("g2", [L, D, 1], F32, isOutput=False)
    b2 = nc.declare_dram_parameter("b2", [L, D, 1], F32, isOutput=False)
    mqd = nc.declare_dram_parameter("mqd", [4, 128, 512], BF16, isOutput=False)
    mlast = nc.declare_dram_parameter("mlast", [128, 128], BF16, isOutput=False)

    SD = 1056                 # downloaded token count (>= 1041)
    kout = nc.declare_dram_parameter("kout", [L, NHC * HD, SD], BF16, isOutput=True)
    vout = nc.declare_dram_parameter("vout", [L, SD, NHC * HD], BF16, isOutput=True)
    xlast = nc.declare_dram_parameter("xlast", [D, 128], BF16, isOutput=True)

    RG = [[0, 1, 2, 3], [4, 5, 6, 7]]
    NBLK = [(0, 512), (512, 512), (1024, 128)]  # free-dim blocks over SP

    with tile.TileContext(nc) as tc:
        with tc.tile_pool(name="pw", bufs=1) as pw, \
             tc.tile_pool(name="px", bufs=1) as px, \
             tc.tile_pool(name="pkv", bufs=1) as pkv, \
             tc.tile_pool(name="pdram", bufs=2, space="DRAM") as pdram:

            # ---- broadcast zero-uploaded params on-device (upload compression) ----
            RG_PAIR = [[0, 4], [1, 5], [2, 6], [3, 7]]

            def bcast(param3, lrows, cols, rg, tag, nl=L):
                ai = pdram.tile([nl * lrows, cols], BF16, name="ai", tag=f"bi{tag}")
                ao = pdram.tile([nl * lrows, cols], BF16, name="ao", tag=f"bo{tag}")
                for l in range(nl):
                    nc.sync.dma_start(ai[l * lrows:(l + 1) * lrows, :],
                                      param3[l, :, :] if nl > 1 else param3[:, :])
                nc.gpsimd.collective_compute("AllReduce", ALU.add, replica_groups=rg,
                                             ins=[ai.opt()], outs=[ao.opt()])
                return ao

            class _P:
                """Read-through view mapping flat [l*rows+r, :] slices to the 3D param."""
                def __init__(self, t, lrows):
                    self.t, self.lrows = t, lrows
                def __getitem__(self, sl):
                    rows = sl[0] if isinstance(sl, tuple) else sl
                    l, r0 = divmod(rows.start, self.lrows)
                    return self.t[l, r0:r0 + (rows.stop - rows.start), :]

            class _X:
                def __getitem__(self, sl):
                    return xt_in[sl]

            BC = os.environ.get("KERNEL_BCAST", "none")
            if BC in ("all", "w"):
                wqk_d = bcast(wqk, D, 2 * NHC * HDP, RG_PAIR, "wqk")
                wv_d = bcast(wv, D, NHC * HD, RG_PAIR, "wv")
                wo_d = bcast(wo, NHC * HDP, D, RG_PAIR, "wo")
                wf1_d = bcast(wf1, D, FFC, RG_PAIR, "wf1")
                wf2_d = bcast(wf2, FFC, D, RG_PAIR, "wf2")
            else:
                wqk_d, wv_d, wo_d = _P(wqk, D), _P(wv, D), _P(wo, NHC * HDP)
                wf1_d, wf2_d = _P(wf1, D), _P(wf2, FFC)
            if BC in ("all", "xt"):
                xt_d = bcast(xt_in, D, SP, RG, "xt", nl=1)
            else:
                xt_d = _X()

            # ---- persistent SBUF: weights, consts, x ----
            w_qk = [[pw.tile([128, 2 * NHC * HDP], BF16, name="w_qk", tag=f"wqk{l}_{k}")
                     for k in range(DK)] for l in range(L)]
            w_v = [[pw.tile([128, NHC * HD], BF16, name="w_v", tag=f"wv{l}_{k}")
                    for k in range(DK)] for l in range(L)]
            w_o = [[pw.tile([128, D], BF16, name="w_o", tag=f"wo{l}_{k}")
                    for k in range(2)] for l in range(L)]
            w_f1 = [[pw.tile([128, FFC], BF16, name="w_f1", tag=f"wf1{l}_{k}")
                     for k in range(DK)] for l in range(L)]
            w_f2 = [[pw.tile([128, D], BF16, name="w_f2", tag=f"wf2{l}_{k}")
                     for k in range(DK)] for l in range(L)]
            for l in range(L):
                for k in range(DK):
                    nc.sync.dma_start(w_qk[l][k][:], wqk_d[l * D + k * 128:l * D + (k + 1) * 128, :])
                    nc.sync.dma_start(w_v[l][k][:], wv_d[l * D + k * 128:l * D + (k + 1) * 128, :])
                    nc.sync.dma_start(w_f1[l][k][:], wf1_d[l * D + k * 128:l * D + (k + 1) * 128, :])
                    nc.sync.dma_start(w_f2[l][k][:], wf2_d[l * FFC + k * 128:l * FFC + (k + 1) * 128, :])
                for k in range(2):
                    nc.sync.dma_start(w_o[l][k][:], wo_d[l * 256 + k * 128:l * 256 + (k + 1) * 128, :])
            bias_v = [pw.tile([1, NHC * HD], BF16, name="bias_v", tag=f"bv{l}") for l in range(L)]
            qkb_sb = pw.tile([128, L * 4], F32, name="qkb_sb")
            ob_sb = pw.tile([128, L * DK], F32, name="ob_sb")
            f1b_sb = pw.tile([128, L * DK], F32, name="f1b_sb")
            f2b_sb = pw.tile([128, L * DK], F32, name="f2b_sb")
            g1a = pw.tile([128, L * DK], F32, name="g1a")
            b1a = pw.tile([128, L * DK], F32, name="b1a")
            g2a = pw.tile([128, L * DK], F32, name="g2a")
            b2a = pw.tile([128, L * DK], F32, name="b2a")
            for l in range(L):
                nc.sync.dma_start(bias_v[l][:], vb[l])
                for m in range(4):
                    nc.sync.dma_start(qkb_sb[:, l * 4 + m:l * 4 + m + 1], qkb[l, m * 128:(m + 1) * 128, :])
                for m in range(DK):
                    nc.sync.dma_start(ob_sb[:, l * DK + m:l * DK + m + 1], ob[l, m * 128:(m + 1) * 128, :])
                    nc.sync.dma_start(f1b_sb[:, l * DK + m:l * DK + m + 1], f1b[l, m * 128:(m + 1) * 128, :])
                    nc.sync.dma_start(f2b_sb[:, l * DK + m:l * DK + m + 1], f2b[l, m * 128:(m + 1) * 128, :])
                for k in range(DK):
                    c = l * DK + k
                    nc.sync.dma_start(g1a[:, c:c + 1], g1[l, k * 128:(k + 1) * 128, :])
                    nc.sync.dma_start(b1a[:, c:c + 1], b1[l, k * 128:(k + 1) * 128, :])
                    nc.sync.dma_start(g2a[:, c:c + 1], g2[l, k * 128:(k + 1) * 128, :])
                    nc.sync.dma_start(b2a[:, c:c + 1], b2[l, k * 128:(k + 1) * 128, :])
            m_qd = [pw.tile([128, 512], BF16, name="m_qd", tag=f"mqd{o}") for o in range(4)]
            m_last = pw.tile([128, 128], BF16, name="m_last")
            for o in range(4):
                nc.sync.dma_start(m_qd[o][:], mqd[o])
            nc.sync.dma_start(m_last[:], mlast[:])
            ones_cb = pw.tile([128, 1], BF16, name="ones_cb")      # partition-sum lhsT (bf16 rhs)
            ones_mb = pw.tile([1, 128], BF16, name="ones_mb")      # bias-trick lhsT (row)
            ones_f = pw.tile([1, 128], F32, name="ones_f")         # K=1 bcast lhsT (f32 rhs)
            eps_t = pw.tile([1, 1], F32, name="eps_t")
            nc.vector.memset(ones_cb[:], 1.0)
            nc.vector.memset(ones_mb[:], 1.0)
            nc.vector.memset(ones_f[:], 1.0)
            nc.vector.memset(eps_t[:], 1e-5)

            # x resident fp32 (residual stream), from bf16 input
            x_f = [px.tile([128, SP], F32, name="x_f", tag=f"x{k}") for k in range(DK)]
            for k in range(DK):
                nc.gpsimd.dma_start(x_f[k][:], xt_d[k * 128:(k + 1) * 128, :])

            # K / V caches (resident for output; also future on-device decode)
            kT = [[pkv.tile([128, SP], BF16, name="kT", tag=f"kT{l}_{i}")
                   for i in range(2)] for l in range(L)]
            vn = [pkv.tile([128, NT * NHC * HD], BF16, name="vn", tag=f"vn{l}")
                  for l in range(L)]  # block st at cols [st*192, st*192+192)

            # ---------------- helper: layernorm ----------------
            def layer_norm(lidx, ga, ba, out_tag):
                """x_f (fp32, 6x[128,SP]) -> bf16 normalized tiles."""
                ht = [pwork1.tile([128, SP], BF16, name="ht", tag=f"ht{k}") for k in range(DK)]
                xb = [pwork1.tile([128, SP], BF16, name="xb", tag=f"wk{k}") for k in range(DK)]
                for k in range(DK):
                    nc.vector.tensor_copy(xb[k][:], x_f[k][:])
                for nb, (c0, cn) in enumerate(NBLK):
                    with tc.tile_pool(name=f"ps_ln{lidx}_{out_tag}_{nb}", bufs=1, space="PSUM") as psl:
                        s_ps = psl.tile([1, 512], F32, name="s_ps", tag="s")
                        q_ps = psl.tile([1, 512], F32, name="q_ps", tag="q")
                        for k in range(DK):
                            sq = pwork2.tile([128, 512], BF16, name="sq", tag="lnsq")
                            nc.scalar.activation(sq[:, :cn], xb[k][:, c0:c0 + cn], AF.Square)
                            nc.tensor.matmul(s_ps[:, :cn], lhsT=ones_cb[:], rhs=xb[k][:, c0:c0 + cn],
                                             start=(k == 0), stop=(k == DK - 1))
                            nc.tensor.matmul(q_ps[:, :cn], lhsT=ones_cb[:], rhs=sq[:, :cn],
                                             start=(k == 0), stop=(k == DK - 1))
                        mn = pstat.tile([1, 512], BF16, name="mn", tag="lnmn")
                        rs = pstat.tile([1, 512], F32, name="rs", tag="lnrs")
                        vr = pstat.tile([1, 512], F32, name="vr", tag="lnvr")
                        nc.vector.tensor_scalar_mul(mn[:, :cn], s_ps[:, :cn], 1.0 / D)
                        nc.vector.tensor_scalar_mul(vr[:, :cn], q_ps[:, :cn], 1.0 / D)
                        # var = E[x^2] - mean^2 ; rstd = 1/sqrt(var+eps)
                        nc.scalar.activation(rs[:, :cn], mn[:, :cn], AF.Square)
                        nc.vector.tensor_sub(vr[:, :cn], vr[:, :cn], rs[:, :cn])
                        nc.scalar.activation(vr[:, :cn], vr[:, :cn], AF.Sqrt, bias=eps_t[:])
                        nc.vector.reciprocal(rs[:, :cn], vr[:, :cn])
                        mb_ps = psl.tile([128, 512], F32, name="mb_ps", tag="mb")
                        rb_ps = psl.tile([128, 512], F32, name="rb_ps", tag="rb")
                        nc.tensor.matmul(mb_ps[:, :cn], lhsT=ones_mb[:], rhs=mn[:, :cn], start=True, stop=True)
                        nc.tensor.matmul(rb_ps[:, :cn], lhsT=ones_f[:], rhs=rs[:, :cn], start=True, stop=True)
                        for k in range(DK):
                            tf = pwork2.tile([128, 512], BF16, name="tf", tag="lntf")
                            nc.vector.tensor_sub(tf[:, :cn], x_f[k][:, c0:c0 + cn], mb_ps[:, :cn])
                            nc.vector.tensor_mul(tf[:, :cn], tf[:, :cn], rb_ps[:, :cn])
                            cc = lidx * DK + k
                            nc.scalar.activation(ht[k][:, c0:c0 + cn], tf[:, :cn], AF.Identity,
                                                 bias=ba[:, cc:cc + 1], scale=ga[:, cc:cc + 1])
                return ht

            # ---------------- helper: AllReduce + residual add ----------------
            def allreduce_add(lidx, ya, tag):
                """ya: 6 bf16 [128,SP] partial tiles -> AR over group -> x_f += result."""
                arin = pdram.tile([D, SP], BF16, name="arin", tag=f"ari{tag}")
                arout = pdram.tile([D, SP], BF16, name="arout", tag=f"aro{tag}")
                for m in range(DK):
                    nc.sync.dma_start(arin[m * 128:(m + 1) * 128, :], ya[m][:])
                nc.gpsimd.collective_compute(
                    "AllReduce", ALU.add, replica_groups=RG,
                    ins=[arin.opt()], outs=[arout.opt()],
                )
                for k in range(DK):
                    ab = pwork1.tile([128, SP], BF16, name="ab", tag="ya0")
                    nc.sync.dma_start(ab[:], arout[k * 128:(k + 1) * 128, :])
                    nc.vector.tensor_add(x_f[k][:], x_f[k][:], ab[:])

            with tc.tile_pool(name="pwork1", bufs=1) as pwork1, \
                 tc.tile_pool(name="pwork2", bufs=2) as pwork2, \
                 tc.tile_pool(name="pstat", bufs=1) as pstat:
                L_EFF = int(os.environ.get("KERNEL_LAYERS", str(L)))
                SKIP_ATTN = bool(os.environ.get("KERNEL_SKIP_ATTN"))
                SKIP_LN = bool(os.environ.get("KERNEL_SKIP_LN"))
                SKIP_GEMM = bool(os.environ.get("KERNEL_SKIP_GEMM"))
                def layer_norm_fake(lidx, ga, ba, out_tag):
                    ht = [pwork1.tile([128, SP], BF16, name="ht", tag=f"ht{k}") for k in range(DK)]
                    for k in range(DK):
                        nc.vector.tensor_copy(ht[k][:], x_f[k][:])
                    return ht
                if SKIP_LN:
                    layer_norm = layer_norm_fake
                for l in range(L_EFF):
                    # ===== ln1 =====
                    ht = layer_norm(l, g1a, b1a, "h1")

                    # ===== qkv GEMM: qkT = [q(256) | k(256)] x SP =====
                    qT = [pwork1.tile([128, SP], BF16, name="qT", tag=f"wk{i}") for i in range(2)]
                    with tc.tile_pool(name=f"ps_qkv{l}", bufs=3, space="PSUM") as psq:
                        for m in range(4):
                            dst = qT[m] if m < 2 else kT[l][m - 2]
                            bc = l * 4 + m
                            for (c0, cn) in NBLK:
                                acc = psq.tile([128, 512], F32, name="acc", tag="acc")
                                for k in range(DK):
                                    nc.tensor.matmul(acc[:, :cn], lhsT=w_qk[l][k][:, m * 128:(m + 1) * 128],
                                                     rhs=ht[k][:, c0:c0 + cn],
                                                     start=(k == 0), stop=(k == DK - 1))
                                nc.scalar.activation(dst[:, c0:c0 + cn], acc[:, :cn], AF.Identity,
                                                     bias=qkb_sb[:, bc:bc + 1])
                        # ===== V in token-major: vn[st] = [128 tokens, 192] =====
                        for st in range(NT):
                            vacc = psq.tile([128, NHC * HD], F32, name="vacc", tag="vacc")
                            for k in range(DK):
                                nc.tensor.matmul(vacc[:], lhsT=ht[k][:, st * 128:(st + 1) * 128],
                                                 rhs=w_v[l][k][:], start=(k == 0), stop=False)
                            nc.tensor.matmul(vacc[:], lhsT=ones_mb[:], rhs=bias_v[l][:],
                                             start=False, stop=True)
                            nc.vector.tensor_copy(vn[l][:, st * 192:(st + 1) * 192], vacc[:])
                    for h in range(NHC):
                        nc.sync.dma_start(kout[l, h * HD:(h + 1) * HD, :],
                                          kT[l][h // 2][64 * (h % 2):64 * (h % 2) + HD, :SD])
                    for st in range(NT):
                        rn = min(128, SD - st * 128)
                        if rn > 0:
                            nc.sync.dma_start(vout[l, st * 128:st * 128 + rn, :],
                                              vn[l][:rn, st * 192:(st + 1) * 192])

                    # ===== attention -> ctxT [256, SP] bf16 =====
                    ctxT = [pwork1.tile([128, SP], BF16, name="ctxT", tag=f"wk{i + 2}") for i in range(2)]
                    for i in range(2):
                        nc.vector.memset(ctxT[i][:], 0.0)
                    QBLK = [(0, 512, 3), (512, 512, 7), (1024, 128, 8)]  # (col0, width, jmax)
                    with tc.tile_pool(name=f"ps_att{l}", bufs=2, space="PSUM") as psa:
                        if SKIP_ATTN:
                            QBLK = []
                        for (c0, cn, jmax) in QBLK:
                            t0 = c0 // 128
                            for h in range(NHC):
                                htile, hrow = h // 2, 64 * (h % 2)
                                cp = psa.tile([HD, 512], F32, name="cp", tag="cp")
                                dp = psa.tile([1, 512], F32, name="dp", tag="dp")
                                for j in range(jmax + 1):
                                    sp = psa.tile([128, 512], F32, name="sp", tag="sp")
                                    nc.tensor.matmul(sp[:, :cn],
                                                     lhsT=kT[l][htile][hrow:hrow + 64, j * 128:(j + 1) * 128],
                                                     rhs=qT[htile][hrow:hrow + 64, c0:c0 + cn],
                                                     start=True, stop=True)
                                    if j >= t0:
                                        msk = m_last if cn == 128 else m_qd[j - t0]
                                        nc.vector.tensor_add(sp[:, :cn], sp[:, :cn], msk[:, :cn])
                                    et = pwork2.tile([128, 512], BF16, name="et", tag="et")
                                    nc.scalar.activation(et[:, :cn], sp[:, :cn], AF.Exp)
                                    nc.tensor.matmul(dp[:, :cn], lhsT=ones_cb[:], rhs=et[:, :cn],
                                                     start=(j == 0), stop=(j == jmax))
                                    nc.tensor.matmul(cp[:, :cn], lhsT=vn[l][:, j * 192 + h * HD:j * 192 + (h + 1) * HD],
                                                     rhs=et[:, :cn], start=(j == 0), stop=(j == jmax))
                                dr = pwork2.tile([1, 512], F32, name="dr", tag="dr")
                                nc.vector.reciprocal(dr[:, :cn], dp[:, :cn])
                                rb = psa.tile([HD, 512], F32, name="rb", tag="rb")
                                nc.tensor.matmul(rb[:, :cn], lhsT=ones_f[:, :HD], rhs=dr[:, :cn], start=True, stop=True)
                                rbs = pwork2.tile([HD, 512], F32, name="rbs", tag="rbs")
                                nc.scalar.copy(rbs[:, :cn], rb[:, :cn])
                                nc.vector.tensor_mul(
                                    ctxT[htile][hrow:hrow + HD, c0:c0 + cn], cp[:, :cn], rbs[:, :cn])

                    # ===== out-proj (row-parallel) + AR + residual =====
                    ya = [pwork1.tile([128, SP], BF16, name="ya", tag=f"ya{m % 3}") for m in range(DK)]
                    with tc.tile_pool(name=f"ps_out{l}", bufs=3, space="PSUM") as pso:
                        for m in range(DK):
                            for (c0, cn) in NBLK:
                                acc = pso.tile([128, 512], F32, name="acc", tag="acc")
                                for k in range(2):
                                    nc.tensor.matmul(acc[:, :cn], lhsT=w_o[l][k][:, m * 128:(m + 1) * 128],
                                                     rhs=ctxT[k][:, c0:c0 + cn], start=(k == 0), stop=(k == 1))
                                bc = l * DK + m
                                nc.scalar.activation(ya[m][:, c0:c0 + cn], acc[:, :cn], AF.Identity,
                                                     bias=ob_sb[:, bc:bc + 1])
                    allreduce_add(l, ya, f"o{l}")

                    # ===== ln2 + ff1 + relu =====
                    h2 = layer_norm(l, g2a, b2a, "h2")
                    fT = [pwork1.tile([128, SP], BF16, name="fT", tag=f"wk{m}") for m in range(DK)]
                    with tc.tile_pool(name=f"ps_ff1{l}", bufs=3, space="PSUM") as psf:
                        for m in range(DK):
                            for (c0, cn) in NBLK:
                                acc = psf.tile([128, 512], F32, name="acc", tag="acc")
                                for k in range(DK):
                                    nc.tensor.matmul(acc[:, :cn], lhsT=w_f1[l][k][:, m * 128:(m + 1) * 128],
                                                     rhs=h2[k][:, c0:c0 + cn], start=(k == 0), stop=(k == DK - 1))
                                bc = l * DK + m
                                nc.scalar.activation(fT[m][:, c0:c0 + cn], acc[:, :cn], AF.Relu,
                                                     bias=f1b_sb[:, bc:bc + 1])
                    # ===== ff2 (row-parallel) + AR + residual =====
                    ya2 = [pwork1.tile([128, SP], BF16, name="ya2", tag=f"ya{(m + 3) % 3}") for m in range(DK)]
                    with tc.tile_pool(name=f"ps_ff2{l}", bufs=3, space="PSUM") as psg:
                        for m in range(DK):
                            for (c0, cn) in NBLK:
                                acc = psg.tile([128, 512], F32, name="acc", tag="acc")
                                for k in range(DK):
                                    nc.tensor.matmul(acc[:, :cn], lhsT=w_f2[l][k][:, m * 128:(m + 1) * 128],
                                                     rhs=fT[k][:, c0:c0 + cn], start=(k == 0), stop=(k == DK - 1))
                                bc = l * DK + m
                                nc.scalar.activation(ya2[m][:, c0:c0 + cn], acc[:, :cn], AF.Identity,
                                                     bias=f2b_sb[:, bc:bc + 1])
                    allreduce_add(l, ya2, f"f{l}")

                # final: export last x tile columns (token 1040 lives at col 1024+16)
                for k in range(DK):
                    nc.gpsimd.dma_start(xlast[k * 128:(k + 1) * 128, :], x_f[k][:, 1024:1152])

    nc.compile()
    _cache["neff"] = nc
    return nc


# ---------------------------------------------------------------------------
# warm runner: build the sharded executable once, keep inputs device-resident,
# execute many times. run_bass_kernel_spmd builds a fresh jax.jit closure per
# call, so every launch would re-link/load the NEFF on top of executing it.
# ---------------------------------------------------------------------------

class WarmRunner:
    def __init__(self, nc, in_maps, n_cores):
        import jax
        import jax.numpy as jnp
        from jax.sharding import Mesh, PartitionSpec, NamedSharding
        from jax.experimental.shard_map import shard_map
        from concourse.bass2jax import (
            _bass_exec_p, partition_id_tensor, install_neuronx_cc_hook,
        )
        self._jax, self._np = jax, np
        install_neuronx_cc_hook()
        partition_name = (
            nc.partition_id_tensor.name if nc.partition_id_tensor else None
        )
        in_names, out_names, out_avals, zero_shapes = [], [], [], []
        for alloc in nc.m.functions[0].allocations:
            if not isinstance(alloc, mybir.MemoryLocationSet):
                continue
            name = alloc.memorylocations[0].name
            if alloc.kind == "ExternalInput":
                if name != partition_name:
                    in_names.append(name)
            elif alloc.kind == "ExternalOutput":
                shape = tuple(alloc.tensor_shape)
                dtype = mybir.dt.np(alloc.dtype)
                out_names.append(name)
                out_avals.append(jax.core.ShapedArray(shape, dtype))
                zero_shapes.append((shape, dtype))
        n_params = len(in_names)
        n_outs = len(out_avals)
        all_in_names = list(in_names) + list(out_names)
        if partition_name is not None:
            all_in_names.append(partition_name)
        donate = tuple(range(n_params, n_params + n_outs))

        def _body(*args):
            operands = list(args)
            if partition_name is not None:
                operands.append(partition_id_tensor())
            outs = _bass_exec_p.bind(
                *operands,
                out_avals=tuple(out_avals),
                in_names=tuple(all_in_names),
                out_names=tuple(out_names),
                lowering_input_output_aliases=(),
                sim_require_finite=True,
                sim_require_nnan=True,
                nc=nc,
            )
            return tuple(outs)

        devices = jax.devices()[:n_cores]
        mesh = Mesh(np.asarray(devices), ("core",))
        in_specs = (PartitionSpec("core"),) * (n_params + n_outs)
        out_specs = (PartitionSpec("core"),) * n_outs
        self._sharded = jax.jit(
            shard_map(_body, mesh=mesh, in_specs=in_specs,
                      out_specs=out_specs, check_rep=False),
            donate_argnums=donate,
            keep_unused=True,
        )
        sh = NamedSharding(mesh, PartitionSpec("core"))
        concat_in = [
            np.concatenate([np.asarray(m[nm]) for m in in_maps], axis=0)
            for nm in in_names
        ]
        self._dev_in = [jax.device_put(a, sh) for a in concat_in]
        jax.block_until_ready(self._dev_in)

        def _mk_zeros():
            return tuple(
                jnp.zeros((n_cores * s[0], *s[1:]), d) for (s, d) in zero_shapes
            )

        self._mk_zeros = jax.jit(_mk_zeros, out_shardings=(sh,) * n_outs)
        self._out_names = out_names
        self._out_avals = out_avals
        self._n_cores = n_cores

    def run_raw(self):
        zeros = self._mk_zeros()
        self._jax.block_until_ready(zeros)
        outs = self._sharded(*self._dev_in, *zeros)
        self._jax.block_until_ready(outs)
        return outs

    def time_ns(self, reps=8, warmup=2):
        import time as _t
        for _ in range(warmup):
            self.run_raw()
        ts = []
        for _ in range(reps):
            zeros = self._mk_zeros()
            self._jax.block_until_ready(zeros)
            t0 = _t.perf_counter_ns()
            outs = self._sharded(*self._dev_in, *zeros)
            self._jax.block_until_ready(outs)
            ts.append(_t.perf_counter_ns() - t0)
        return ts

    def results(self):
        outs = self.run_raw()
        res = []
        for c in range(self._n_cores):
            m = {}
            for i, nm in enumerate(self._out_names):
                shp = self._out_avals[i].shape
                m[nm] = np.asarray(outs[i]).reshape(self._n_cores, *shp)[c]
            res.append(m)
        return res


# ---------------------------------------------------------------------------
# host side
# ---------------------------------------------------------------------------

def _ln_np(x, g, b, eps=1e-5):
    m = x.mean(-1, keepdims=True)
    v = ((x - m) ** 2).mean(-1, keepdims=True)
    return (x - m) / np.sqrt(v + eps) * g + b


def _gelu(z):
    from scipy.special import erf
    return 0.5 * z * (1 + erf(z / np.sqrt(2)))


def _prep_seq0(i):
    img = i['feats'].transpose(0, 2, 1) + i['img_pos_enc']
    it = i['intent_embeds'][np.clip(i['intent'].astype(np.int64) - 1, 0, 2)][:, None, :]
    past = i['past'] @ i['past_W'] + i['past_b'] + i['time_embeds'][:N_PAST]
    return np.concatenate([img, it, past], 1).astype(np.float32)  # [B, 1041, D]


def _make_in_maps(i):
    seq0 = _prep_seq0(i)
    sc = 1.0 / np.sqrt(HD)
    masks = {}
    # scoresT layout: ROW = key, COL = query -> allowed iff key <= query.
    # m_qd[o]: mask for key-tile at diagonal offset o within a 512-wide q block
    # (q tiles t: t<o fully masked, t==o triangular, t>o unmasked).
    r = np.arange(128)
    tri = r[:, None] <= r[None, :]
    md = np.zeros((4, 128, 512), np.float32)
    for o in range(4):
        for t in range(4):
            blk = md[o][:, t * 128:(t + 1) * 128]
            if t < o:
                blk[:] = NEG
            elif t == o:
                blk[:] = np.where(tri, 0.0, NEG)
    mdq = md.astype(NPBF16)
    ml = np.where(tri & (r[:, None] <= 16), 0.0, NEG).astype(NPBF16)
    in_maps = []
    for core in range(NC):
        g, c = divmod(core, TP)
        heads = range(NHC * c, NHC * (c + 1))
        xt = np.zeros((D, SP), np.float32)
        xt[:, :S0] = seq0[g].T
        # --- weight slices, head-padded to HDP ---
        wqk = np.zeros((L, D, 2 * NHC * HDP), np.float32)
        qkbias = np.zeros((L, 1, 2 * NHC * HDP), np.float32)  # transposed at pack time
        wvv = np.zeros((L, D, NHC * HD), np.float32)
        vbias = np.zeros((L, 1, NHC * HD), np.float32)
        wob = np.zeros((L, NHC * HDP, D), np.float32)
        for l in range(L):
            Wq = i['qkv_W'][l][:, :D]
            Wk = i['qkv_W'][l][:, D:2 * D]
            Wv = i['qkv_W'][l][:, 2 * D:]
            bq = i['qkv_b'][l][:D]
            bk = i['qkv_b'][l][D:2 * D]
            bv = i['qkv_b'][l][2 * D:]
            Wo = i['out_W'][l]
            for hi, h in enumerate(heads):
                cs = slice(h * HD, (h + 1) * HD)
                wqk[l, :, hi * HDP:hi * HDP + HD] = Wq[:, cs] * sc
                qkbias[l, 0, hi * HDP:hi * HDP + HD] = bq[cs] * sc
                wqk[l, :, NHC * HDP + hi * HDP:NHC * HDP + hi * HDP + HD] = Wk[:, cs]
                qkbias[l, 0, NHC * HDP + hi * HDP:NHC * HDP + hi * HDP + HD] = bk[cs]
                wvv[l, :, hi * HD:(hi + 1) * HD] = Wv[:, cs]
                vbias[l, 0, hi * HD:(hi + 1) * HD] = bv[cs]
                wob[l, hi * HDP:hi * HDP + HD, :] = Wo[cs, :]
        f1s = slice(c * FFC, (c + 1) * FFC)
        bcm = os.environ.get("KERNEL_BCAST", "none")
        zw = (core >= TP) and bcm in ("all", "w")
        zx = (c != 0) and bcm in ("all", "xt")
        z16 = lambda a: np.zeros_like(a) if zw else a
        im = {
            "xt": np.zeros((D, SP), NPBF16) if zx else xt.astype(NPBF16),
            "wqk": z16(wqk.astype(NPBF16)),
            "wv": z16(wvv.astype(NPBF16)),
            "wo": z16(wob.astype(NPBF16)),
            "wf1": z16(np.ascontiguousarray(i['ff1_W'][:, :, f1s]).astype(NPBF16)),
            "wf2": z16(np.ascontiguousarray(i['ff2_W'][:, f1s, :]).astype(NPBF16)),
            "qkb": qkbias.transpose(0, 2, 1).astype(np.float32),
            "vb": vbias.astype(NPBF16),
            "ob": (i['out_b'][:, :, None] if c == 0 else np.zeros((L, D, 1))).astype(np.float32),
            "f1b": np.ascontiguousarray(i['ff1_b'][:, f1s, None]).astype(np.float32),
            "f2b": (i['ff2_b'][:, :, None] if c == 0 else np.zeros((L, D, 1))).astype(np.float32),
            "g1": np.ascontiguousarray(i['ln1_g'][:, :, None]).astype(np.float32),
            "b1": np.ascontiguousarray(i['ln1_b'][:, :, None]).astype(np.float32),
            "g2": np.ascontiguousarray(i['ln2_g'][:, :, None]).astype(np.float32),
            "b2": np.ascontiguousarray(i['ln2_b'][:, :, None]).astype(np.float32),
            "mqd": mdq,
            "mlast": ml,
        }
        in_maps.append(im)
    return in_maps


def _host_decode(i, K, V, x_final):
    """20 AR steps with device-filled KV caches. K,V: [L,B,SMAX,H,HD] fp32."""
    preds = []
    h_out = x_final[:, None, :]                      # [B,1,D]
    for t in range(T_DEC):
        if t > 0:
            pos = S0 + t - 1                         # index of the new token
            x = nxt
            for l in range(L):
                h = _ln_np(x, i['ln1_g'][l], i['ln1_b'][l])
                qkv = h @ i['qkv_W'][l] + i['qkv_b'][l]
                q, k, v = np.split(qkv, 3, -1)
                K[l][:, pos] = k.reshape(B, H, HD)
                V[l][:, pos] = v.reshape(B, H, HD)
                q = q.reshape(B, 1, H, HD)
                kk = K[l][:, :pos + 1]
                vv = V[l][:, :pos + 1]
                s = np.einsum('bqhd,bkhd->bhqk', q, kk) / np.sqrt(HD)
                e = np.exp(s - s.max(-1, keepdims=True))
                a = e / e.sum(-1, keepdims=True)
                ctx = np.einsum('bhqk,bkhd->bqhd', a, vv).reshape(B, 1, H * HD)
                x = x + ctx @ i['out_W'][l] + i['out_b'][l]
                h2 = _ln_np(x, i['ln2_g'][l], i['ln2_b'][l])
                x = x + np.maximum(h2 @ i['ff1_W'][l] + i['ff1_b'][l], 0) @ i['ff2_W'][l] + i['ff2_b'][l]
            h_out = x
        p = _gelu(h_out @ i['dec1_W'] + i['dec1_b']) @ i['dec2_W'] + i['dec2_b']
        preds.append(p)
        nxt = (p @ i['pos_W'] + i['pos_b'] + i['time_embeds'][N_PAST + t]).astype(np.float32)
    return np.concatenate(preds, 1).astype(np.float32)


def kernel(**inputs):
    import time as _time
    t0 = _time.perf_counter()
    i = {k: np.asarray(v) for k, v in inputs.items()}
    if os.environ.get("KERNEL_HOST_ONLY"):
        return _host_forward_ref(i)
    nc = _build_prefill_neff()
    t1 = _time.perf_counter()
    in_maps = _make_in_maps(i)
    t2 = _time.perf_counter()
    res = run_bass_kernel_spmd(nc, in_maps, core_ids=list(range(NC)), trace=False).results
    t3 = _time.perf_counter()

    SMAX = S0 + T_DEC
    K = [np.zeros((B, SMAX, H, HD), np.float32) for _ in range(L)]
    V = [np.zeros((B, SMAX, H, HD), np.float32) for _ in range(L)]
    x_final = np.zeros((B, D), np.float32)
    for core in range(NC):
        g, c = divmod(core, TP)
        ko = res[core]["kout"].astype(np.float32)   # [L, 192, 1056]
        vo = res[core]["vout"].astype(np.float32)   # [L, 1056, 192]
        for l in range(L):
            for hi in range(NHC):
                h = NHC * c + hi
                K[l][g, :S0, h, :] = ko[l, hi * HD:(hi + 1) * HD, :S0].T
                V[l][g, :S0, h, :] = vo[l, :S0, hi * HD:(hi + 1) * HD]
        if c == 0:
            x_final[g] = res[core]["xlast"].astype(np.float32)[:, 16]
    t4 = _time.perf_counter()
    out = _host_decode(i, K, V, x_final)
    t5 = _time.perf_counter()
    if os.environ.get("KERNEL_TIMING"):
        print(f"[kernel] compile={t1 - t0:.2f}s prep={t2 - t1:.2f}s launch={t3 - t2:.2f}s "
              f"extract={t4 - t3:.2f}s decode={t5 - t4:.2f}s", flush=True)
    return out


def _host_forward_ref(i):
    """Pure-host fallback (debug): full KV-cache forward in numpy fp32."""
    seq0 = _prep_seq0(i)
    SMAX = S0 + T_DEC
    K = [np.zeros((B, SMAX, H, HD), np.float32) for _ in range(L)]
    V = [np.zeros((B, SMAX, H, HD), np.float32) for _ in range(L)]
    x = seq0
    for l in range(L):
        h = _ln_np(x, i['ln1_g'][l], i['ln1_b'][l])
        qkv = h @ i['qkv_W'][l] + i['qkv_b'][l]
        q, k, v = np.split(qkv, 3, -1)
        T = x.shape[1]
        K[l][:, :T] = k.reshape(B, T, H, HD)
        V[l][:, :T] = v.reshape(B, T, H, HD)
        q = q.reshape(B, T, H, HD)
        s = np.einsum('bqhd,bkhd->bhqk', q, K[l][:, :T]) / np.sqrt(HD)
        mask = np.tril(np.ones((T, T), dtype=bool))
        s = np.where(mask[None, None], s, -np.inf)
        e = np.exp(s - s.max(-1, keepdims=True))
        a = e / e.sum(-1, keepdims=True)
        ctx = np.einsum('bhqk,bkhd->bqhd', a, V[l][:, :T]).reshape(B, T, H * HD)
        x = x + ctx @ i['out_W'][l] + i['out_b'][l]
        h2 = _ln_np(x, i['ln2_g'][l], i['ln2_b'][l])
        x = x + np.maximum(h2 @ i['ff1_W'][l] + i['ff1_b'][l], 0) @ i['ff2_W'][l] + i['ff2_b'][l]
    return _host_decode(i, K, V, x[:, -1, :])



# revision 9
# speedup vs baseline: 940.7735x; 940.7735x over previous
"""nn_ARMonocularModel: KV-cache AR transformer on 8 TRN2 NeuronCores.

Reformulation: the reference recomputes the full causal encoder each of the
20 AR steps; under causal masking that equals prefill-once + KV-cache decode
(earlier token activations are step-invariant). 2.19 TFLOP -> ~0.11 TFLOP.

Device: ONE SPMD NEFF over 8 cores runs the full 3-layer prefill
(S=1041 tokens, B=2). Sharding = data-parallel over batch (cores 0-3 get
batch 0, cores 4-7 batch 1) x Megatron tensor-parallel over the 16 heads
(4/core) and the 3072 FFN cols (768/core) within each group, with bf16
AllReduce after out-proj and ff2 (replica groups [[0..3],[4..7]]).

Layout: activations live TRANSPOSED (xT [768, S]) so every GEMM is
weight-stationary (lhsT = W-tile) with xT as the moving operand, and
attention needs no transposes at all:
  scoresT[k,q] = kT-block.T @ qT   (per head; K-dim = head_dim pad 64)
  softmax along the PARTITION (key) axis via ones-matmul column sums
  (no max subtraction -- |scores| < 2 by construction, verified)
  ctxT[48,q]   = Vn-block.T @ expT (Vn = V in token-major layout)
LayerNorm stats (over D = partition axis) also use ones-matmul sums.
Biases are folded in as K=1 rank-1 matmuls (ones-row trick).

The 20-step serial decode (2 tokens/step) is latency-bound and runs on
host fp32 from the downloaded K/V caches; its FLOPs are negligible.
"""
import os
import numpy as np
import ml_dtypes

import concourse.bass as bass
import concourse.mybir as mybir
import concourse.tile as tile
from concourse import bacc
from concourse.bass_utils import run_bass_kernel_spmd

F32 = mybir.dt.float32
BF16 = mybir.dt.bfloat16
AF = mybir.ActivationFunctionType
ALU = mybir.AluOpType
NPBF16 = ml_dtypes.bfloat16

NC = 8
TP = 4                      # tensor-parallel group size
B, D, H, HD, L = 2, 768, 16, 48, 3
HDP = 64                    # head dim padded (zero cols) so 4 heads = 256 rows
NHC = H // TP               # heads per core = 4
N_IMG, N_PAST, T_DEC = 1024, 16, 20
S0 = N_IMG + 1 + N_PAST     # 1041
SP = 1152                   # padded tokens = 9 tiles of 128
NT = SP // 128              # 9
DK = D // 128               # 6
FFC = 4 * D // TP           # ff1 cols per core = 768
NEG = -30000.0

_cache = {}
import os as _os


def _build_prefill_neff(repeat=1):
    """repeat>1 builds a timing variant: the full prefill body unrolled
    `repeat` times in one NEFF (re-reads xt each iteration). Used by the
    bench harness to amortize per-launch overhead; kernel() uses repeat=1."""
    key = f"neff{repeat}"
    if key in _cache:
        return _cache[key]
    nc = bacc.Bacc("TRN2", target_bir_lowering=False, debug=False, num_devices=NC)

    # ---- DRAM params (per-core) ----
    xt_in = nc.declare_dram_parameter("xt", [D, SP], BF16, isOutput=False)
    wqk = nc.declare_dram_parameter("wqk", [L, D, 2 * NHC * HDP], BF16, isOutput=False)
    wv = nc.declare_dram_parameter("wv", [L, D, NHC * HD], BF16, isOutput=False)
    wo = nc.declare_dram_parameter("wo", [L, NHC * HDP, D], BF16, isOutput=False)
    wf1 = nc.declare_dram_parameter("wf1", [L, D, FFC], BF16, isOutput=False)
    wf2 = nc.declare_dram_parameter("wf2", [L, FFC, D], BF16, isOutput=False)
    qkb = nc.declare_dram_parameter("qkb", [L, 2 * NHC * HDP, 1], F32, isOutput=False)
    vb = nc.declare_dram_parameter("vb", [L, 1, NHC * HD], BF16, isOutput=False)
    ob = nc.declare_dram_parameter("ob", [L, D, 1], F32, isOutput=False)
    f1b = nc.declare_dram_parameter("f1b", [L, FFC, 1], F32, isOutput=False)
    f2b = nc.declare_dram_parameter("f2b", [L, D, 1], F32, isOutput=False)
    g1 = nc.declare_dram_parameter("g1", [L, D, 1], F32, isOutput=False)
    b1 = nc.declare_dram_parameter("b1", [L, D, 1], F32, isOutput=False)
    g2 = nc.declare_dram_parameter("g2", [L, D, 1], F32, isOutput=False)
    b2 = nc.declare_dram_parameter("b2", [L, D, 1], F32, isOutput=False)
    mqd = nc.declare_dram_parameter("mqd", [4, 128, 512], BF16, isOutput=False)
    mlast = nc.declare_dram_parameter("mlast", [128, 128], BF16, isOutput=False)

    SD = 1056                 # downloaded token count (>= 1041)
    kout = nc.declare_dram_parameter("kout", [L, NHC * HD, SD], BF16, isOutput=True)
    vout = nc.declare_dram_parameter("vout", [L, SD, NHC * HD], BF16, isOutput=True)
    xlast = nc.declare_dram_parameter("xlast", [D, 128], BF16, isOutput=True)

    RG = [[0, 1, 2, 3], [4, 5, 6, 7]]
    NBLK = [(0, 512), (512, 512), (1024, 128)]  # free-dim blocks over SP

    with tile.TileContext(nc) as tc:
        with tc.tile_pool(name="pw", bufs=1) as pw, \
             tc.tile_pool(name="px", bufs=1) as px, \
             tc.tile_pool(name="pkv", bufs=1) as pkv, \
             tc.tile_pool(name="pdram", bufs=2, space="DRAM") as pdram:

            # ---- broadcast zero-uploaded params on-device (upload compression) ----
            RG_PAIR = [[0, 4], [1, 5], [2, 6], [3, 7]]

            def bcast(param3, lrows, cols, rg, tag, nl=L):
                ai = pdram.tile([nl * lrows, cols], BF16, name="ai", tag=f"bi{tag}")
                ao = pdram.tile([nl * lrows, cols], BF16, name="ao", tag=f"bo{tag}")
                for l in range(nl):
                    nc.sync.dma_start(ai[l * lrows:(l + 1) * lrows, :],
                                      param3[l, :, :] if nl > 1 else param3[:, :])
                nc.gpsimd.collective_compute("AllReduce", ALU.add, replica_groups=rg,
                                             ins=[ai.opt()], outs=[ao.opt()])
                return ao

            class _P:
                """Read-through view mapping flat [l*rows+r, :] slices to the 3D param."""
                def __init__(self, t, lrows):
                    self.t, self.lrows = t, lrows
                def __getitem__(self, sl):
                    rows = sl[0] if isinstance(sl, tuple) else sl
                    l, r0 = divmod(rows.start, self.lrows)
                    return self.t[l, r0:r0 + (rows.stop - rows.start), :]

            class _X:
                def __getitem__(self, sl):
                    return xt_in[sl]

            BC = os.environ.get("KERNEL_BCAST", "none")
            if BC in ("all", "w"):
                wqk_d = bcast(wqk, D, 2 * NHC * HDP, RG_PAIR, "wqk")
                wv_d = bcast(wv, D, NHC * HD, RG_PAIR, "wv")
                wo_d = bcast(wo, NHC * HDP, D, RG_PAIR, "wo")
                wf1_d = bcast(wf1, D, FFC, RG_PAIR, "wf1")
                wf2_d = bcast(wf2, FFC, D, RG_PAIR, "wf2")
            else:
                wqk_d, wv_d, wo_d = _P(wqk, D), _P(wv, D), _P(wo, NHC * HDP)
                wf1_d, wf2_d = _P(wf1, D), _P(wf2, FFC)
            if BC in ("all", "xt"):
                xt_d = bcast(xt_in, D, SP, RG, "xt", nl=1)
            else:
                xt_d = _X()

            # ---- persistent SBUF: weights, consts, x ----
            w_qk = [[pw.tile([128, 2 * NHC * HDP], BF16, name="w_qk", tag=f"wqk{l}_{k}")
                     for k in range(DK)] for l in range(L)]
            w_v = [[pw.tile([128, NHC * HD], BF16, name="w_v", tag=f"wv{l}_{k}")
                    for k in range(DK)] for l in range(L)]
            w_o = [[pw.tile([128, D], BF16, name="w_o", tag=f"wo{l}_{k}")
                    for k in range(2)] for l in range(L)]
            w_f1 = [[pw.tile([128, FFC], BF16, name="w_f1", tag=f"wf1{l}_{k}")
                     for k in range(DK)] for l in range(L)]
            w_f2 = [[pw.tile([128, D], BF16, name="w_f2", tag=f"wf2{l}_{k}")
                     for k in range(DK)] for l in range(L)]
            for l in range(L):
                for k in range(DK):
                    nc.sync.dma_start(w_qk[l][k][:], wqk_d[l * D + k * 128:l * D + (k + 1) * 128, :])
                    nc.sync.dma_start(w_v[l][k][:], wv_d[l * D + k * 128:l * D + (k + 1) * 128, :])
                    nc.sync.dma_start(w_f1[l][k][:], wf1_d[l * D + k * 128:l * D + (k + 1) * 128, :])
                    nc.sync.dma_start(w_f2[l][k][:], wf2_d[l * FFC + k * 128:l * FFC + (k + 1) * 128, :])
                for k in range(2):
                    nc.sync.dma_start(w_o[l][k][:], wo_d[l * 256 + k * 128:l * 256 + (k + 1) * 128, :])
            bias_v = [pw.tile([1, NHC * HD], BF16, name="bias_v", tag=f"bv{l}") for l in range(L)]
            qkb_sb = pw.tile([128, L * 4], F32, name="qkb_sb")
            ob_sb = pw.tile([128, L * DK], F32, name="ob_sb")
            f1b_sb = pw.tile([128, L * DK], F32, name="f1b_sb")
            f2b_sb = pw.tile([128, L * DK], F32, name="f2b_sb")
            g1a = pw.tile([128, L * DK], F32, name="g1a")
            b1a = pw.tile([128, L * DK], F32, name="b1a")
            g2a = pw.tile([128, L * DK], F32, name="g2a")
            b2a = pw.tile([128, L * DK], F32, name="b2a")
            for l in range(L):
                nc.sync.dma_start(bias_v[l][:], vb[l])
                for m in range(4):
                    nc.sync.dma_start(qkb_sb[:, l * 4 + m:l * 4 + m + 1], qkb[l, m * 128:(m + 1) * 128, :])
                for m in range(DK):
                    nc.sync.dma_start(ob_sb[:, l * DK + m:l * DK + m + 1], ob[l, m * 128:(m + 1) * 128, :])
                    nc.sync.dma_start(f1b_sb[:, l * DK + m:l * DK + m + 1], f1b[l, m * 128:(m + 1) * 128, :])
                    nc.sync.dma_start(f2b_sb[:, l * DK + m:l * DK + m + 1], f2b[l, m * 128:(m + 1) * 128, :])
                for k in range(DK):
                    c = l * DK + k
                    nc.sync.dma_start(g1a[:, c:c + 1], g1[l, k * 128:(k + 1) * 128, :])
                    nc.sync.dma_start(b1a[:, c:c + 1], b1[l, k * 128:(k + 1) * 128, :])
                    nc.sync.dma_start(g2a[:, c:c + 1], g2[l, k * 128:(k + 1) * 128, :])
                    nc.sync.dma_start(b2a[:, c:c + 1], b2[l, k * 128:(k + 1) * 128, :])
            m_qd = [pw.tile([128, 512], BF16, name="m_qd", tag=f"mqd{o}") for o in range(4)]
            m_last = pw.tile([128, 128], BF16, name="m_last")
            for o in range(4):
                nc.sync.dma_start(m_qd[o][:], mqd[o])
            nc.sync.dma_start(m_last[:], mlast[:])
            ones_cb = pw.tile([128, 1], BF16, name="ones_cb")      # partition-sum lhsT (bf16 rhs)
            ones_mb = pw.tile([1, 128], BF16, name="ones_mb")      # bias-trick lhsT (row)
            ones_f = pw.tile([1, 128], F32, name="ones_f")         # K=1 bcast lhsT (f32 rhs)
            eps_t = pw.tile([1, 1], F32, name="eps_t")
            nc.vector.memset(ones_cb[:], 1.0)
            nc.vector.memset(ones_mb[:], 1.0)
            nc.vector.memset(ones_f[:], 1.0)
            nc.vector.memset(eps_t[:], 1e-5)

            # x resident fp32 (residual stream), from bf16 input
            x_f = [px.tile([128, SP], F32, name="x_f", tag=f"x{k}") for k in range(DK)]

            # K / V caches (resident for output; also future on-device decode)
            kT = [[pkv.tile([128, SP], BF16, name="kT", tag=f"kT{l}_{i}")
                   for i in range(2)] for l in range(L)]
            vn = [pkv.tile([128, NT * NHC * HD], BF16, name="vn", tag=f"vn{l}")
                  for l in range(L)]  # block st at cols [st*192, st*192+192)

            # ---------------- helper: layernorm ----------------
            def layer_norm(lidx, ga, ba, out_tag):
                """x_f (fp32, 6x[128,SP]) -> bf16 normalized tiles."""
                ht = [pwork1.tile([128, SP], BF16, name="ht", tag=f"ht{k}") for k in range(DK)]
                xb = [pwork1.tile([128, SP], BF16, name="xb", tag=f"wk{k}") for k in range(DK)]
                for k in range(DK):
                    nc.vector.tensor_copy(xb[k][:], x_f[k][:])
                for nb, (c0, cn) in enumerate(NBLK):
                    with tc.tile_pool(name=f"ps_ln{lidx}_{out_tag}_{nb}", bufs=1, space="PSUM") as psl:
                        s_ps = psl.tile([1, 512], F32, name="s_ps", tag="s")
                        q_ps = psl.tile([1, 512], F32, name="q_ps", tag="q")
                        for k in range(DK):
                            sq = pwork2.tile([128, 512], BF16, name="sq", tag="lnsq")
                            nc.scalar.activation(sq[:, :cn], xb[k][:, c0:c0 + cn], AF.Square)
                            nc.tensor.matmul(s_ps[:, :cn], lhsT=ones_cb[:], rhs=xb[k][:, c0:c0 + cn],
                                             start=(k == 0), stop=(k == DK - 1))
                            nc.tensor.matmul(q_ps[:, :cn], lhsT=ones_cb[:], rhs=sq[:, :cn],
                                             start=(k == 0), stop=(k == DK - 1))
                        mn = pstat.tile([1, 512], BF16, name="mn", tag="lnmn")
                        rs = pstat.tile([1, 512], F32, name="rs", tag="lnrs")
                        vr = pstat.tile([1, 512], F32, name="vr", tag="lnvr")
                        nc.vector.tensor_scalar_mul(mn[:, :cn], s_ps[:, :cn], 1.0 / D)
                        nc.vector.tensor_scalar_mul(vr[:, :cn], q_ps[:, :cn], 1.0 / D)
                        # var = E[x^2] - mean^2 ; rstd = 1/sqrt(var+eps)
                        nc.scalar.activation(rs[:, :cn], mn[:, :cn], AF.Square)
                        nc.vector.tensor_sub(vr[:, :cn], vr[:, :cn], rs[:, :cn])
                        nc.scalar.activation(vr[:, :cn], vr[:, :cn], AF.Sqrt, bias=eps_t[:])
                        nc.vector.reciprocal(rs[:, :cn], vr[:, :cn])
                        mb_ps = psl.tile([128, 512], F32, name="mb_ps", tag="mb")
                        rb_ps = psl.tile([128, 512], F32, name="rb_ps", tag="rb")
                        nc.tensor.matmul(mb_ps[:, :cn], lhsT=ones_mb[:], rhs=mn[:, :cn], start=True, stop=True)
                        nc.tensor.matmul(rb_ps[:, :cn], lhsT=ones_f[:], rhs=rs[:, :cn], start=True, stop=True)
                        for k in range(DK):
                            tf = pwork2.tile([128, 512], BF16, name="tf", tag="lntf")
                            nc.vector.tensor_sub(tf[:, :cn], x_f[k][:, c0:c0 + cn], mb_ps[:, :cn])
                            nc.vector.tensor_mul(tf[:, :cn], tf[:, :cn], rb_ps[:, :cn])
                            cc = lidx * DK + k
                            nc.scalar.activation(ht[k][:, c0:c0 + cn], tf[:, :cn], AF.Identity,
                                                 bias=ba[:, cc:cc + 1], scale=ga[:, cc:cc + 1])
                return ht

            # ---------------- helper: AllReduce + residual add ----------------
            def allreduce_add(lidx, ya, tag):
                """ya: 6 bf16 [128,SP] partial tiles -> AR over group -> x_f += result."""
                arin = pdram.tile([D, SP], BF16, name="arin", tag=f"ari{tag}")
                arout = pdram.tile([D, SP], BF16, name="arout", tag=f"aro{tag}")
                for m in range(DK):
                    nc.sync.dma_start(arin[m * 128:(m + 1) * 128, :], ya[m][:])
                nc.gpsimd.collective_compute(
                    "AllReduce", ALU.add, replica_groups=RG,
                    ins=[arin.opt()], outs=[arout.opt()],
                )
                for k in range(DK):
                    ab = pwork1.tile([128, SP], BF16, name="ab", tag="ya0")
                    nc.sync.dma_start(ab[:], arout[k * 128:(k + 1) * 128, :])
                    nc.vector.tensor_add(x_f[k][:], x_f[k][:], ab[:])

            with tc.tile_pool(name="pwork1", bufs=1) as pwork1, \
                 tc.tile_pool(name="pwork2", bufs=2) as pwork2, \
                 tc.tile_pool(name="pstat", bufs=1) as pstat:
                L_EFF = int(os.environ.get("KERNEL_LAYERS", str(L)))
                SKIP_ATTN = bool(os.environ.get("KERNEL_SKIP_ATTN"))
                SKIP_LN = bool(os.environ.get("KERNEL_SKIP_LN"))
                SKIP_GEMM = bool(os.environ.get("KERNEL_SKIP_GEMM"))
                def layer_norm_fake(lidx, ga, ba, out_tag):
                    ht = [pwork1.tile([128, SP], BF16, name="ht", tag=f"ht{k}") for k in range(DK)]
                    for k in range(DK):
                        nc.vector.tensor_copy(ht[k][:], x_f[k][:])
                    return ht
                if SKIP_LN:
                    layer_norm = layer_norm_fake
                for r, l in [(r, l) for r in range(repeat) for l in range(L_EFF)]:
                    rl = f"{r}_{l}" if repeat > 1 else str(l)
                    if l == 0:
                        for k in range(DK):
                            nc.gpsimd.dma_start(x_f[k][:], xt_d[k * 128:(k + 1) * 128, :])
                    # ===== ln1 =====
                    ht = layer_norm(l, g1a, b1a, f"h1{rl}")

                    # ===== qkv GEMM: qkT = [q(256) | k(256)] x SP =====
                    qT = [pwork1.tile([128, SP], BF16, name="qT", tag=f"wk{i}") for i in range(2)]
                    with tc.tile_pool(name=f"ps_qkv{rl}", bufs=3, space="PSUM") as psq:
                        for m in range(4):
                            dst = qT[m] if m < 2 else kT[l][m - 2]
                            bc = l * 4 + m
                            for (c0, cn) in NBLK:
                                acc = psq.tile([128, 512], F32, name="acc", tag="acc")
                                for k in range(DK):
                                    nc.tensor.matmul(acc[:, :cn], lhsT=w_qk[l][k][:, m * 128:(m + 1) * 128],
                                                     rhs=ht[k][:, c0:c0 + cn],
                                                     start=(k == 0), stop=(k == DK - 1))
                                nc.scalar.activation(dst[:, c0:c0 + cn], acc[:, :cn], AF.Identity,
                                                     bias=qkb_sb[:, bc:bc + 1])
                        # ===== V in token-major: vn[st] = [128 tokens, 192] =====
                        for st in range(NT):
                            vacc = psq.tile([128, NHC * HD], F32, name="vacc", tag="vacc")
                            for k in range(DK):
                                nc.tensor.matmul(vacc[:], lhsT=ht[k][:, st * 128:(st + 1) * 128],
                                                 rhs=w_v[l][k][:], start=(k == 0), stop=False)
                            nc.tensor.matmul(vacc[:], lhsT=ones_mb[:], rhs=bias_v[l][:],
                                             start=False, stop=True)
                            nc.vector.tensor_copy(vn[l][:, st * 192:(st + 1) * 192], vacc[:])
                    for h in range(NHC):
                        nc.sync.dma_start(kout[l, h * HD:(h + 1) * HD, :],
                                          kT[l][h // 2][64 * (h % 2):64 * (h % 2) + HD, :SD])
                    for st in range(NT):
                        rn = min(128, SD - st * 128)
                        if rn > 0:
                            nc.sync.dma_start(vout[l, st * 128:st * 128 + rn, :],
                                              vn[l][:rn, st * 192:(st + 1) * 192])

                    # ===== attention -> ctxT [256, SP] bf16 =====
                    ctxT = [pwork1.tile([128, SP], BF16, name="ctxT", tag=f"wk{i + 2}") for i in range(2)]
                    for i in range(2):
                        nc.vector.memset(ctxT[i][:], 0.0)
                    QBLK = [(0, 512, 3), (512, 512, 7), (1024, 128, 8)]  # (col0, width, jmax)
                    with tc.tile_pool(name=f"ps_att{rl}", bufs=2, space="PSUM") as psa:
                        if SKIP_ATTN:
                            QBLK = []
                        for (c0, cn, jmax) in QBLK:
                            t0 = c0 // 128
                            for h in range(NHC):
                                htile, hrow = h // 2, 64 * (h % 2)
                                cp = psa.tile([HD, 512], F32, name="cp", tag="cp")
                                dp = psa.tile([1, 512], F32, name="dp", tag="dp")
                                for j in range(jmax + 1):
                                    sp = psa.tile([128, 512], F32, name="sp", tag="sp")
                                    nc.tensor.matmul(sp[:, :cn],
                                                     lhsT=kT[l][htile][hrow:hrow + 64, j * 128:(j + 1) * 128],
                                                     rhs=qT[htile][hrow:hrow + 64, c0:c0 + cn],
                                                     start=True, stop=True)
                                    if j >= t0:
                                        msk = m_last if cn == 128 else m_qd[j - t0]
                                        nc.vector.tensor_add(sp[:, :cn], sp[:, :cn], msk[:, :cn])
                                    et = pwork2.tile([128, 512], BF16, name="et", tag="et")
                                    nc.scalar.activation(et[:, :cn], sp[:, :cn], AF.Exp)
                                    nc.tensor.matmul(dp[:, :cn], lhsT=ones_cb[:], rhs=et[:, :cn],
                                                     start=(j == 0), stop=(j == jmax))
                                    nc.tensor.matmul(cp[:, :cn], lhsT=vn[l][:, j * 192 + h * HD:j * 192 + (h + 1) * HD],
                                                     rhs=et[:, :cn], start=(j == 0), stop=(j == jmax))
                                dr = pwork2.tile([1, 512], F32, name="dr", tag="dr")
                                nc.vector.reciprocal(dr[:, :cn], dp[:, :cn])
                                rb = psa.tile([HD, 512], F32, name="rb", tag="rb")
                                nc.tensor.matmul(rb[:, :cn], lhsT=ones_f[:, :HD], rhs=dr[:, :cn], start=True, stop=True)
                                rbs = pwork2.tile([HD, 512], F32, name="rbs", tag="rbs")
                                nc.scalar.copy(rbs[:, :cn], rb[:, :cn])
                                nc.vector.tensor_mul(
                                    ctxT[htile][hrow:hrow + HD, c0:c0 + cn], cp[:, :cn], rbs[:, :cn])

                    # ===== out-proj (row-parallel) + AR + residual =====
                    ya = [pwork1.tile([128, SP], BF16, name="ya", tag=f"ya{m % 3}") for m in range(DK)]
                    with tc.tile_pool(name=f"ps_out{rl}", bufs=3, space="PSUM") as pso:
                        for m in range(DK):
                            for (c0, cn) in NBLK:
                                acc = pso.tile([128, 512], F32, name="acc", tag="acc")
                                for k in range(2):
                                    nc.tensor.matmul(acc[:, :cn], lhsT=w_o[l][k][:, m * 128:(m + 1) * 128],
                                                     rhs=ctxT[k][:, c0:c0 + cn], start=(k == 0), stop=(k == 1))
                                bc = l * DK + m
                                nc.scalar.activation(ya[m][:, c0:c0 + cn], acc[:, :cn], AF.Identity,
                                                     bias=ob_sb[:, bc:bc + 1])
                    allreduce_add(l, ya, f"o{rl}")

                    # ===== ln2 + ff1 + relu =====
                    h2 = layer_norm(l, g2a, b2a, f"h2{rl}")
                    fT = [pwork1.tile([128, SP], BF16, name="fT", tag=f"wk{m}") for m in range(DK)]
                    with tc.tile_pool(name=f"ps_ff1{rl}", bufs=3, space="PSUM") as psf:
                        for m in range(DK):
                            for (c0, cn) in NBLK:
                                acc = psf.tile([128, 512], F32, name="acc", tag="acc")
                                for k in range(DK):
                                    nc.tensor.matmul(acc[:, :cn], lhsT=w_f1[l][k][:, m * 128:(m + 1) * 128],
                                                     rhs=h2[k][:, c0:c0 + cn], start=(k == 0), stop=(k == DK - 1))
                                bc = l * DK + m
                                nc.scalar.activation(fT[m][:, c0:c0 + cn], acc[:, :cn], AF.Relu,
                                                     bias=f1b_sb[:, bc:bc + 1])
                    # ===== ff2 (row-parallel) + AR + residual =====
                    ya2 = [pwork1.tile([128, SP], BF16, name="ya2", tag=f"ya{(m + 3) % 3}") for m in range(DK)]
                    with tc.tile_pool(name=f"ps_ff2{rl}", bufs=3, space="PSUM") as psg:
                        for m in range(DK):
                            for (c0, cn) in NBLK:
                                acc = psg.tile([128, 512], F32, name="acc", tag="acc")
                                for k in range(DK):
                                    nc.tensor.matmul(acc[:, :cn], lhsT=w_f2[l][k][:, m * 128:(m + 1) * 128],
                                                     rhs=fT[k][:, c0:c0 + cn], start=(k == 0), stop=(k == DK - 1))
                                bc = l * DK + m
                                nc.scalar.activation(ya2[m][:, c0:c0 + cn], acc[:, :cn], AF.Identity,
                                                     bias=f2b_sb[:, bc:bc + 1])
                    allreduce_add(l, ya2, f"f{rl}")

                # final: export last x tile columns (token 1040 lives at col 1024+16)
                for k in range(DK):
                    nc.gpsimd.dma_start(xlast[k * 128:(k + 1) * 128, :], x_f[k][:, 1024:1152])

    nc.compile()
    _cache["neff"] = nc
    return nc


# ---------------------------------------------------------------------------
# warm runner: build the sharded executable once, keep inputs device-resident,
# execute many times. run_bass_kernel_spmd builds a fresh jax.jit closure per
# call, so every launch would re-link/load the NEFF on top of executing it.
# ---------------------------------------------------------------------------

class WarmRunner:
    def __init__(self, nc, in_maps, n_cores):
        import jax
        import jax.numpy as jnp
        from jax.sharding import Mesh, PartitionSpec, NamedSharding
        from jax.experimental.shard_map import shard_map
        from concourse.bass2jax import (
            _bass_exec_p, partition_id_tensor, install_neuronx_cc_hook,
        )
        self._jax, self._np = jax, np
        install_neuronx_cc_hook()
        partition_name = (
            nc.partition_id_tensor.name if nc.partition_id_tensor else None
        )
        in_names, out_names, out_avals, zero_shapes = [], [], [], []
        for alloc in nc.m.functions[0].allocations:
            if not isinstance(alloc, mybir.MemoryLocationSet):
                continue
            name = alloc.memorylocations[0].name
            if alloc.kind == "ExternalInput":
                if name != partition_name:
                    in_names.append(name)
            elif alloc.kind == "ExternalOutput":
                shape = tuple(alloc.tensor_shape)
                dtype = mybir.dt.np(alloc.dtype)
                out_names.append(name)
                out_avals.append(jax.core.ShapedArray(shape, dtype))
                zero_shapes.append((shape, dtype))
        n_params = len(in_names)
        n_outs = len(out_avals)
        all_in_names = list(in_names) + list(out_names)
        if partition_name is not None:
            all_in_names.append(partition_name)
        donate = tuple(range(n_params, n_params + n_outs))

        def _body(*args):
            operands = list(args)
            if partition_name is not None:
                operands.append(partition_id_tensor())
            outs = _bass_exec_p.bind(
                *operands,
                out_avals=tuple(out_avals),
                in_names=tuple(all_in_names),
                out_names=tuple(out_names),
                lowering_input_output_aliases=(),
                sim_require_finite=True,
                sim_require_nnan=True,
                nc=nc,
            )
            return tuple(outs)

        devices = jax.devices()[:n_cores]
        mesh = Mesh(np.asarray(devices), ("core",))
        in_specs = (PartitionSpec("core"),) * (n_params + n_outs)
        out_specs = (PartitionSpec("core"),) * n_outs
        self._sharded = jax.jit(
            shard_map(_body, mesh=mesh, in_specs=in_specs,
                      out_specs=out_specs, check_rep=False),
            donate_argnums=donate,
            keep_unused=True,
        )
        sh = NamedSharding(mesh, PartitionSpec("core"))
        concat_in = [
            np.concatenate([np.asarray(m[nm]) for m in in_maps], axis=0)
            for nm in in_names
        ]
        self._dev_in = [jax.device_put(a, sh) for a in concat_in]
        jax.block_until_ready(self._dev_in)

        def _mk_zeros():
            return tuple(
                jnp.zeros((n_cores * s[0], *s[1:]), d) for (s, d) in zero_shapes
            )

        self._mk_zeros = jax.jit(_mk_zeros, out_shardings=(sh,) * n_outs)
        self._out_names = out_names
        self._out_avals = out_avals
        self._n_cores = n_cores

    def run_raw(self):
        zeros = self._mk_zeros()
        self._jax.block_until_ready(zeros)
        outs = self._sharded(*self._dev_in, *zeros)
        self._jax.block_until_ready(outs)
        return outs

    def time_ns(self, reps=8, warmup=2):
        import time as _t
        for _ in range(warmup):
            self.run_raw()
        ts = []
        for _ in range(reps):
            zeros = self._mk_zeros()
            self._jax.block_until_ready(zeros)
            t0 = _t.perf_counter_ns()
            outs = self._sharded(*self._dev_in, *zeros)
            self._jax.block_until_ready(outs)
            ts.append(_t.perf_counter_ns() - t0)
        return ts

    def results(self):
        outs = self.run_raw()
        res = []
        for c in range(self._n_cores):
            m = {}
            for i, nm in enumerate(self._out_names):
                shp = self._out_avals[i].shape
                m[nm] = np.asarray(outs[i]).reshape(self._n_cores, *shp)[c]
            res.append(m)
        return res


# ---------------------------------------------------------------------------
# host side
# ---------------------------------------------------------------------------

def _ln_np(x, g, b, eps=1e-5):
    m = x.mean(-1, keepdims=True)
    v = ((x - m) ** 2).mean(-1, keepdims=True)
    return (x - m) / np.sqrt(v + eps) * g + b


def _gelu(z):
    from scipy.special import erf
    return 0.5 * z * (1 + erf(z / np.sqrt(2)))


def _prep_seq0(i):
    img = i['feats'].transpose(0, 2, 1) + i['img_pos_enc']
    it = i['intent_embeds'][np.clip(i['intent'].astype(np.int64) - 1, 0, 2)][:, None, :]
    past = i['past'] @ i['past_W'] + i['past_b'] + i['time_embeds'][:N_PAST]
    return np.concatenate([img, it, past], 1).astype(np.float32)  # [B, 1041, D]


def _make_in_maps(i):
    seq0 = _prep_seq0(i)
    sc = 1.0 / np.sqrt(HD)
    masks = {}
    # scoresT layout: ROW = key, COL = query -> allowed iff key <= query.
    # m_qd[o]: mask for key-tile at diagonal offset o within a 512-wide q block
    # (q tiles t: t<o fully masked, t==o triangular, t>o unmasked).
    r = np.arange(128)
    tri = r[:, None] <= r[None, :]
    md = np.zeros((4, 128, 512), np.float32)
    for o in range(4):
        for t in range(4):
            blk = md[o][:, t * 128:(t + 1) * 128]
            if t < o:
                blk[:] = NEG
            elif t == o:
                blk[:] = np.where(tri, 0.0, NEG)
    mdq = md.astype(NPBF16)
    ml = np.where(tri & (r[:, None] <= 16), 0.0, NEG).astype(NPBF16)
    in_maps = []
    for core in range(NC):
        g, c = divmod(core, TP)
        heads = range(NHC * c, NHC * (c + 1))
        xt = np.zeros((D, SP), np.float32)
        xt[:, :S0] = seq0[g].T
        # --- weight slices, head-padded to HDP ---
        wqk = np.zeros((L, D, 2 * NHC * HDP), np.float32)
        qkbias = np.zeros((L, 1, 2 * NHC * HDP), np.float32)  # transposed at pack time
        wvv = np.zeros((L, D, NHC * HD), np.float32)
        vbias = np.zeros((L, 1, NHC * HD), np.float32)
        wob = np.zeros((L, NHC * HDP, D), np.float32)
        for l in range(L):
            Wq = i['qkv_W'][l][:, :D]
            Wk = i['qkv_W'][l][:, D:2 * D]
            Wv = i['qkv_W'][l][:, 2 * D:]
            bq = i['qkv_b'][l][:D]
            bk = i['qkv_b'][l][D:2 * D]
            bv = i['qkv_b'][l][2 * D:]
            Wo = i['out_W'][l]
            for hi, h in enumerate(heads):
                cs = slice(h * HD, (h + 1) * HD)
                wqk[l, :, hi * HDP:hi * HDP + HD] = Wq[:, cs] * sc
                qkbias[l, 0, hi * HDP:hi * HDP + HD] = bq[cs] * sc
                wqk[l, :, NHC * HDP + hi * HDP:NHC * HDP + hi * HDP + HD] = Wk[:, cs]
                qkbias[l, 0, NHC * HDP + hi * HDP:NHC * HDP + hi * HDP + HD] = bk[cs]
                wvv[l, :, hi * HD:(hi + 1) * HD] = Wv[:, cs]
                vbias[l, 0, hi * HD:(hi + 1) * HD] = bv[cs]
                wob[l, hi * HDP:hi * HDP + HD, :] = Wo[cs, :]
        f1s = slice(c * FFC, (c + 1) * FFC)
        bcm = os.environ.get("KERNEL_BCAST", "none")
        zw = (core >= TP) and bcm in ("all", "w")
        zx = (c != 0) and bcm in ("all", "xt")
        z16 = lambda a: np.zeros_like(a) if zw else a
        im = {
            "xt": np.zeros((D, SP), NPBF16) if zx else xt.astype(NPBF16),
            "wqk": z16(wqk.astype(NPBF16)),
            "wv": z16(wvv.astype(NPBF16)),
            "wo": z16(wob.astype(NPBF16)),
            "wf1": z16(np.ascontiguousarray(i['ff1_W'][:, :, f1s]).astype(NPBF16)),
            "wf2": z16(np.ascontiguousarray(i['ff2_W'][:, f1s, :]).astype(NPBF16)),
            "qkb": qkbias.transpose(0, 2, 1).astype(np.float32),
            "vb": vbias.astype(NPBF16),
            "ob": (i['out_b'][:, :, None] if c == 0 else np.zeros((L, D, 1))).astype(np.float32),
            "f1b": np.ascontiguousarray(i['ff1_b'][:, f1s, None]).astype(np.float32),
            "f2b": (i['ff2_b'][:, :, None] if c == 0 else np.zeros((L, D, 1))).astype(np.float32),
            "g1": np.ascontiguousarray(i['ln1_g'][:, :, None]).astype(np.float32),
            "b1": np.ascontiguousarray(i['ln1_b'][:, :, None]).astype(np.float32),
            "g2": np.ascontiguousarray(i['ln2_g'][:, :, None]).astype(np.float32),
            "b2": np.ascontiguousarray(i['ln2_b'][:, :, None]).astype(np.float32),
            "mqd": mdq,
            "mlast": ml,
        }
        in_maps.append(im)
    return in_maps


def _host_decode(i, K, V, x_final):
    """20 AR steps with device-filled KV caches. K,V: [L,B,SMAX,H,HD] fp32."""
    preds = []
    h_out = x_final[:, None, :]                      # [B,1,D]
    for t in range(T_DEC):
        if t > 0:
            pos = S0 + t - 1                         # index of the new token
            x = nxt
            for l in range(L):
                h = _ln_np(x, i['ln1_g'][l], i['ln1_b'][l])
                qkv = h @ i['qkv_W'][l] + i['qkv_b'][l]
                q, k, v = np.split(qkv, 3, -1)
                K[l][:, pos] = k.reshape(B, H, HD)
                V[l][:, pos] = v.reshape(B, H, HD)
                q = q.reshape(B, 1, H, HD)
                kk = K[l][:, :pos + 1]
                vv = V[l][:, :pos + 1]
                s = np.einsum('bqhd,bkhd->bhqk', q, kk) / np.sqrt(HD)
                e = np.exp(s - s.max(-1, keepdims=True))
                a = e / e.sum(-1, keepdims=True)
                ctx = np.einsum('bhqk,bkhd->bqhd', a, vv).reshape(B, 1, H * HD)
                x = x + ctx @ i['out_W'][l] + i['out_b'][l]
                h2 = _ln_np(x, i['ln2_g'][l], i['ln2_b'][l])
                x = x + np.maximum(h2 @ i['ff1_W'][l] + i['ff1_b'][l], 0) @ i['ff2_W'][l] + i['ff2_b'][l]
            h_out = x
        p = _gelu(h_out @ i['dec1_W'] + i['dec1_b']) @ i['dec2_W'] + i['dec2_b']
        preds.append(p)
        nxt = (p @ i['pos_W'] + i['pos_b'] + i['time_embeds'][N_PAST + t]).astype(np.float32)
    return np.concatenate(preds, 1).astype(np.float32)


def kernel(**inputs):
    import time as _time
    t0 = _time.perf_counter()
    i = {k: np.asarray(v) for k, v in inputs.items()}
    if os.environ.get("KERNEL_HOST_ONLY"):
        return _host_forward_ref(i)
    nc = _build_prefill_neff()
    t1 = _time.perf_counter()
    in_maps = _make_in_maps(i)
    t2 = _time.perf_counter()
    if os.environ.get("KERNEL_NO_WARM"):
        res = run_bass_kernel_spmd(nc, in_maps, core_ids=list(range(NC)), trace=False).results
    else:
        runner = WarmRunner(nc, in_maps, NC)
        _cache["runner"] = runner
        res = runner.results()
    t3 = _time.perf_counter()

    SMAX = S0 + T_DEC
    K = [np.zeros((B, SMAX, H, HD), np.float32) for _ in range(L)]
    V = [np.zeros((B, SMAX, H, HD), np.float32) for _ in range(L)]
    x_final = np.zeros((B, D), np.float32)
    for core in range(NC):
        g, c = divmod(core, TP)
        ko = res[core]["kout"].astype(np.float32)   # [L, 192, 1056]
        vo = res[core]["vout"].astype(np.float32)   # [L, 1056, 192]
        for l in range(L):
            for hi in range(NHC):
                h = NHC * c + hi
                K[l][g, :S0, h, :] = ko[l, hi * HD:(hi + 1) * HD, :S0].T
                V[l][g, :S0, h, :] = vo[l, :S0, hi * HD:(hi + 1) * HD]
        if c == 0:
            x_final[g] = res[core]["xlast"].astype(np.float32)[:, 16]
    t4 = _time.perf_counter()
    out = _host_decode(i, K, V, x_final)
    t5 = _time.perf_counter()
    if os.environ.get("KERNEL_TIMING"):
        print(f"[kernel] compile={t1 - t0:.2f}s prep={t2 - t1:.2f}s launch={t3 - t2:.2f}s "
              f"extract={t4 - t3:.2f}s decode={t5 - t4:.2f}s", flush=True)
    return out


def _host_forward_ref(i):
    """Pure-host fallback (debug): full KV-cache forward in numpy fp32."""
    seq0 = _prep_seq0(i)
    SMAX = S0 + T_DEC
    K = [np.zeros((B, SMAX, H, HD), np.float32) for _ in range(L)]
    V = [np.zeros((B, SMAX, H, HD), np.float32) for _ in range(L)]
    x = seq0
    for l in range(L):
        h = _ln_np(x, i['ln1_g'][l], i['ln1_b'][l])
        qkv = h @ i['qkv_W'][l] + i['qkv_b'][l]
        q, k, v = np.split(qkv, 3, -1)
        T = x.shape[1]
        K[l][:, :T] = k.reshape(B, T, H, HD)
        V[l][:, :T] = v.reshape(B, T, H, HD)
        q = q.reshape(B, T, H, HD)
        s = np.einsum('bqhd,bkhd->bhqk', q, K[l][:, :T]) / np.sqrt(HD)
        mask = np.tril(np.ones((T, T), dtype=bool))
        s = np.where(mask[None, None], s, -np.inf)
        e = np.exp(s - s.max(-1, keepdims=True))
        a = e / e.sum(-1, keepdims=True)
        ctx = np.einsum('bhqk,bkhd->bqhd', a, V[l][:, :T]).reshape(B, T, H * HD)
        x = x + ctx @ i['out_W'][l] + i['out_b'][l]
        h2 = _ln_np(x, i['ln2_g'][l], i['ln2_b'][l])
        x = x + np.maximum(h2 @ i['ff1_W'][l] + i['ff1_b'][l], 0) @ i['ff2_W'][l] + i['ff2_b'][l]
    return _host_decode(i, K, V, x[:, -1, :])



# revision 24
# speedup vs baseline: 1294.8051x; 1.3763x over previous
"""nn_ARMonocularModel: KV-cache AR transformer on 8 TRN2 NeuronCores.

Reformulation: the reference recomputes the full causal encoder each of the
20 AR steps; under causal masking that equals prefill-once + KV-cache decode
(earlier token activations are step-invariant). 2.19 TFLOP -> ~0.11 TFLOP.

Device: ONE SPMD NEFF over 8 cores runs the full 3-layer prefill
(S=1041 tokens, B=2). Sharding = data-parallel over batch (cores 0-3 get
batch 0, cores 4-7 batch 1) x Megatron tensor-parallel over the 16 heads
(4/core) and the 3072 FFN cols (768/core) within each group, with bf16
AllReduce after out-proj and ff2 (replica groups [[0..3],[4..7]]).

Layout: activations live TRANSPOSED (xT [768, S]) so every GEMM is
weight-stationary (lhsT = W-tile) with xT as the moving operand, and
attention needs no transposes at all:
  scoresT[k,q] = kT-block.T @ qT   (per head; K-dim = head_dim pad 64)
  softmax along the PARTITION (key) axis via ones-matmul column sums
  (no max subtraction -- |scores| < 2 by construction, verified)
  ctxT[48,q]   = Vn-block.T @ expT (Vn = V in token-major layout)
LayerNorm stats (over D = partition axis) also use ones-matmul sums.
Biases are folded in as K=1 rank-1 matmuls (ones-row trick).

The 20-step serial decode (2 tokens/step) is latency-bound and runs on
host fp32 from the downloaded K/V caches; its FLOPs are negligible.
"""
import os
import numpy as np
import ml_dtypes

import concourse.bass as bass
import concourse.mybir as mybir
import concourse.tile as tile
from concourse import bacc
from concourse.bass_utils import run_bass_kernel_spmd

F32 = mybir.dt.float32
BF16 = mybir.dt.bfloat16
AF = mybir.ActivationFunctionType
ALU = mybir.AluOpType
NPBF16 = ml_dtypes.bfloat16

NC = 8
TP = 4                      # tensor-parallel group size
B, D, H, HD, L = 2, 768, 16, 48, 3
HDP = 64                    # head dim padded (zero cols) so 4 heads = 256 rows
NHC = H // TP               # heads per core = 4
N_IMG, N_PAST, T_DEC = 1024, 16, 20
S0 = N_IMG + 1 + N_PAST     # 1041
SP = 1056                   # padded tokens = 8 tiles of 128 + 32
NT = (SP + 127) // 128      # 9 (last tile 32 rows)
SEG = NHC * (HD + 1)        # vn cols per key tile: 4 heads x (48 + ones-col)
DK = D // 128               # 6
FFC = 4 * D // TP           # ff1 cols per core = 768
NEG = -30000.0


def st_h(j, h):
    """vn column offset of head h's (48+1)-col block within key tile j."""
    return j * SEG + h * (HD + 1)

_cache = {}
import os as _os


def _build_prefill_neff(repeat=1):
    """repeat>1 builds a timing variant: the full prefill body unrolled
    `repeat` times in one NEFF (re-reads xt each iteration). Used by the
    bench harness to amortize per-launch overhead; kernel() uses repeat=1."""
    key = f"neff{repeat}"
    if key in _cache:
        return _cache[key]
    nc = bacc.Bacc("TRN2", target_bir_lowering=False, debug=False, num_devices=NC)

    # ---- DRAM params (per-core) ----
    xt_in = nc.declare_dram_parameter("xt", [D, SP], BF16, isOutput=False)
    wqk = nc.declare_dram_parameter("wqk", [L, D, 2 * NHC * HDP], BF16, isOutput=False)
    wv = nc.declare_dram_parameter("wv", [L, D, SEG], BF16, isOutput=False)
    wo = nc.declare_dram_parameter("wo", [L, NHC * HDP, D], BF16, isOutput=False)
    wf1 = nc.declare_dram_parameter("wf1", [L, D, FFC], BF16, isOutput=False)
    wf2 = nc.declare_dram_parameter("wf2", [L, FFC, D], BF16, isOutput=False)
    qkb = nc.declare_dram_parameter("qkb", [L, 2 * NHC * HDP, 1], F32, isOutput=False)
    vb = nc.declare_dram_parameter("vb", [L, 1, SEG], BF16, isOutput=False)
    f1b = nc.declare_dram_parameter("f1b", [L, FFC, 1], F32, isOutput=False)
    f2b = nc.declare_dram_parameter("f2b", [L, D, 1], F32, isOutput=False)
    gb1 = nc.declare_dram_parameter("gb1", [L, 2, D], BF16, isOutput=False)
    gb2 = nc.declare_dram_parameter("gb2", [L, 2, D], BF16, isOutput=False)
    mqd = nc.declare_dram_parameter("mqd", [4, 128, 512], BF16, isOutput=False)
    mlast = nc.declare_dram_parameter("mlast", [32, 32], BF16, isOutput=False)

    SD = 1056                 # downloaded token count (>= 1041)
    kout = nc.declare_dram_parameter("kout", [L, NHC * HD, SD], BF16, isOutput=True)
    vout = nc.declare_dram_parameter("vout", [L, SD, SEG], BF16, isOutput=True)
    xlast = nc.declare_dram_parameter("xlast", [D, 32], BF16, isOutput=True)

    RG = [[0, 1, 2, 3], [4, 5, 6, 7]]
    NBLK = [(0, 512), (512, 512), (1024, 32)]  # free-dim blocks over SP

    with tile.TileContext(nc) as tc:
        with tc.tile_pool(name="pw", bufs=1) as pw, \
             tc.tile_pool(name="px", bufs=1) as px, \
             tc.tile_pool(name="pkv", bufs=1) as pkv, \
             tc.tile_pool(name="pdram", bufs=2, space="DRAM") as pdram:

            # ---- broadcast zero-uploaded params on-device (upload compression) ----
            RG_PAIR = [[0, 4], [1, 5], [2, 6], [3, 7]]

            def bcast(param3, lrows, cols, rg, tag, nl=L):
                ai = pdram.tile([nl * lrows, cols], BF16, name="ai", tag=f"bi{tag}")
                ao = pdram.tile([nl * lrows, cols], BF16, name="ao", tag=f"bo{tag}")
                for l in range(nl):
                    nc.sync.dma_start(ai[l * lrows:(l + 1) * lrows, :],
                                      param3[l, :, :] if nl > 1 else param3[:, :])
                nc.gpsimd.collective_compute("AllReduce", ALU.add, replica_groups=rg,
                                             ins=[ai.opt()], outs=[ao.opt()])
                return ao

            class _P:
                """Read-through view mapping flat [l*rows+r, :] slices to the 3D param."""
                def __init__(self, t, lrows):
                    self.t, self.lrows = t, lrows
                def __getitem__(self, sl):
                    rows = sl[0] if isinstance(sl, tuple) else sl
                    l, r0 = divmod(rows.start, self.lrows)
                    return self.t[l, r0:r0 + (rows.stop - rows.start), :]

            class _X:
                def __getitem__(self, sl):
                    return xt_in[sl]

            BC = os.environ.get("KERNEL_BCAST", "none")
            if BC in ("all", "w"):
                wqk_d = bcast(wqk, D, 2 * NHC * HDP, RG_PAIR, "wqk")
                wv_d = bcast(wv, D, SEG, RG_PAIR, "wv")
                wo_d = bcast(wo, NHC * HDP, D, RG_PAIR, "wo")
                wf1_d = bcast(wf1, D, FFC, RG_PAIR, "wf1")
                wf2_d = bcast(wf2, FFC, D, RG_PAIR, "wf2")
            else:
                wqk_d, wv_d, wo_d = _P(wqk, D), _P(wv, D), _P(wo, NHC * HDP)
                wf1_d, wf2_d = _P(wf1, D), _P(wf2, FFC)
            if BC in ("all", "xt"):
                xt_d = bcast(xt_in, D, SP, RG, "xt", nl=1)
            else:
                xt_d = _X()

            # ---- persistent SBUF: weights, consts, x ----
            w_qk = [[pw.tile([128, 2 * NHC * HDP], BF16, name="w_qk", tag=f"wqk{l}_{k}")
                     for k in range(DK)] for l in range(L)]
            w_v = [[pw.tile([128, SEG], BF16, name="w_v", tag=f"wv{l}_{k}")
                    for k in range(DK)] for l in range(L)]
            w_o = [[pw.tile([128, D], BF16, name="w_o", tag=f"wo{l}_{k}")
                    for k in range(2)] for l in range(L)]
            w_f1 = [[pw.tile([128, FFC], BF16, name="w_f1", tag=f"wf1{l}_{k}")
                     for k in range(DK)] for l in range(L)]
            w_f2 = [[pw.tile([128, D], BF16, name="w_f2", tag=f"wf2{l}_{k}")
                     for k in range(DK)] for l in range(L)]
            for l in range(L):
                for k in range(DK):
                    nc.sync.dma_start(w_qk[l][k][:], wqk_d[l * D + k * 128:l * D + (k + 1) * 128, :])
                    nc.sync.dma_start(w_v[l][k][:], wv_d[l * D + k * 128:l * D + (k + 1) * 128, :])
                    nc.sync.dma_start(w_f1[l][k][:], wf1_d[l * D + k * 128:l * D + (k + 1) * 128, :])
                    nc.sync.dma_start(w_f2[l][k][:], wf2_d[l * FFC + k * 128:l * FFC + (k + 1) * 128, :])
                for k in range(2):
                    nc.sync.dma_start(w_o[l][k][:], wo_d[l * 256 + k * 128:l * 256 + (k + 1) * 128, :])
            bias_v = [pw.tile([1, SEG], BF16, name="bias_v", tag=f"bv{l}") for l in range(L)]
            qkb_sb = pw.tile([128, L * 4], F32, name="qkb_sb")
            f1b_sb = pw.tile([128, L * DK], F32, name="f1b_sb")
            f2b_sb = pw.tile([128, L * DK], F32, name="f2b_sb")
            g1t = pw.tile([1, L * D], BF16, name="g1t")
            b1t = pw.tile([1, L * D], BF16, name="b1t")
            g2t = pw.tile([1, L * D], BF16, name="g2t")
            b2t = pw.tile([1, L * D], BF16, name="b2t")
            ones_row = pw.tile([1, 512], BF16, name="ones_row")
            for l in range(L):
                nc.sync.dma_start(bias_v[l][:], vb[l])
                for m in range(4):
                    nc.sync.dma_start(qkb_sb[:, l * 4 + m:l * 4 + m + 1], qkb[l, m * 128:(m + 1) * 128, :])
                for m in range(DK):
                    nc.sync.dma_start(f1b_sb[:, l * DK + m:l * DK + m + 1], f1b[l, m * 128:(m + 1) * 128, :])
                    nc.sync.dma_start(f2b_sb[:, l * DK + m:l * DK + m + 1], f2b[l, m * 128:(m + 1) * 128, :])
                nc.sync.dma_start(g1t[:, l * D:(l + 1) * D], gb1[l, 0:1, :])
                nc.sync.dma_start(b1t[:, l * D:(l + 1) * D], gb1[l, 1:2, :])
                nc.sync.dma_start(g2t[:, l * D:(l + 1) * D], gb2[l, 0:1, :])
                nc.sync.dma_start(b2t[:, l * D:(l + 1) * D], gb2[l, 1:2, :])
            m_qd = [pw.tile([128, 512], BF16, name="m_qd", tag=f"mqd{o}") for o in range(4)]
            m_last = pw.tile([32, 32], BF16, name="m_last")
            for o in range(4):
                nc.sync.dma_start(m_qd[o][:], mqd[o])
            nc.sync.dma_start(m_last[:], mlast[:])
            ones_cb = pw.tile([128, 1], BF16, name="ones_cb")      # partition-sum lhsT (bf16 rhs)
            ones_mb = pw.tile([1, 128], BF16, name="ones_mb")      # bias-trick lhsT (row)
            eps_t = pw.tile([1, 1], F32, name="eps_t")
            nc.vector.memset(ones_cb[:], 1.0)
            nc.vector.memset(ones_mb[:], 1.0)
            nc.vector.memset(eps_t[:], 1e-5)
            nc.vector.memset(ones_row[:], 1.0)

            # x resident fp32 (residual stream), from bf16 input
            x_f = [px.tile([128, SP], F32, name="x_f", tag=f"x{k}") for k in range(DK)]

            # K / V caches (resident for output; also future on-device decode)
            kT_sets = [[pkv.tile([128, SP], BF16, name="kT", tag=f"kT{p}_{i}")
                        for i in range(2)] for p in range(2)]
            vn_sets = [pkv.tile([128, NT * SEG], BF16, name="vn", tag=f"vn{p}")
                       for p in range(2)]  # block st at cols [st*SEG, (st+1)*SEG)

            # ---------------- helper: layernorm ----------------
            def layer_norm(lidx, gbt, out_tag):
                """x_f (fp32, 6x[128,SP]) -> bf16 ht = x*(g*rstd) + (b - mean*rstd*g).
                Broadcasts over tokens via K<=2 matmuls with [g; b] as lhsT."""
                ht = [pwork1.tile([128, SP], BF16, name="ht", tag=f"ht{k}") for k in range(DK)]
                xb = [pwork1.tile([128, SP], BF16, name="xb", tag=f"wk{k}") for k in range(DK)]
                for k in range(DK):
                    nc.vector.tensor_copy(xb[k][:], x_f[k][:])
                for nb, (c0, cn) in enumerate(NBLK):
                    with tc.tile_pool(name=f"ps_ln{lidx}_{out_tag}_{nb}", bufs=1, space="PSUM") as psl, \
                         tc.tile_pool(name=f"ps_lnb{lidx}_{out_tag}_{nb}", bufs=2, space="PSUM") as pslb:
                        s_ps = psl.tile([1, 512], F32, name="s_ps", tag="s")
                        q_ps = psl.tile([1, 512], F32, name="q_ps", tag="q")
                        for k in range(DK):
                            sq = pwork2.tile([128, 512], BF16, name="sq", tag="w512")
                            nc.scalar.activation(sq[:, :cn], x_f[k][:, c0:c0 + cn], AF.Square)
                            nc.tensor.matmul(s_ps[:, :cn], lhsT=ones_cb[:], rhs=xb[k][:, c0:c0 + cn],
                                             start=(k == 0), stop=(k == DK - 1))
                            nc.tensor.matmul(q_ps[:, :cn], lhsT=ones_cb[:], rhs=sq[:, :cn],
                                             start=(k == 0), stop=(k == DK - 1))
                        mn = pstat.tile([1, 512], F32, name="mn", tag="lnmn")
                        rs = pstat.tile([1, 512], BF16, name="rs", tag="lnrs")
                        vr = pstat.tile([1, 512], F32, name="vr", tag="lnvr")
                        m2 = pstat.tile([1, 512], F32, name="m2", tag="lnm2")
                        nmrs = pstat.tile([1, 512], BF16, name="nmrs", tag="lnnm")
                        nc.vector.tensor_scalar_mul(mn[:, :cn], s_ps[:, :cn], 1.0 / D)
                        nc.vector.tensor_scalar_mul(vr[:, :cn], q_ps[:, :cn], 1.0 / D)
                        # var = E[x^2] - mean^2 ; rstd = 1/sqrt(var+eps)
                        nc.scalar.activation(m2[:, :cn], mn[:, :cn], AF.Square)
                        nc.vector.tensor_sub(vr[:, :cn], vr[:, :cn], m2[:, :cn])
                        nc.scalar.activation(vr[:, :cn], vr[:, :cn], AF.Sqrt, bias=eps_t[:])
                        with nc.allow_low_precision(reason="rstd bf16 broadcast rhs"):
                            nc.vector.reciprocal(rs[:, :cn], vr[:, :cn])
                        # nmrs = -(mean*rstd)
                        nc.vector.scalar_tensor_tensor(nmrs[:, :cn], mn[:, :cn], -1.0, rs[:, :cn],
                                                       op0=ALU.mult, op1=ALU.mult)
                        gt, bt = gbt
                        for k in range(DK):
                            gc = lidx * D + k * 128
                            rbg_ps = pslb.tile([128, 512], F32, name="rbg_ps", tag="rbg")
                            ofs_ps = pslb.tile([128, 512], F32, name="ofs_ps", tag="ofs")
                            nc.tensor.matmul(rbg_ps[:, :cn], lhsT=gt[:, gc:gc + 128], rhs=rs[:, :cn],
                                             start=True, stop=True)
                            # ofs = -(mean*rstd)*g + b
                            nc.tensor.matmul(ofs_ps[:, :cn], lhsT=gt[:, gc:gc + 128], rhs=nmrs[:, :cn],
                                             start=True, stop=False)
                            nc.tensor.matmul(ofs_ps[:, :cn], lhsT=bt[:, gc:gc + 128], rhs=ones_row[:, :cn],
                                             start=False, stop=True)
                            tf = pwork2.tile([128, 512], BF16, name="tf", tag="w512")
                            nc.vector.tensor_mul(tf[:, :cn], x_f[k][:, c0:c0 + cn], rbg_ps[:, :cn])
                            nc.vector.tensor_add(ht[k][:, c0:c0 + cn], tf[:, :cn], ofs_ps[:, :cn])
                return ht

            # ---------------- helper: AllReduce + residual add ----------------
            def allreduce_add(lidx, ya, tag):
                """ya: 6 bf16 [128,SP] partial tiles -> AR over group -> x_f += result."""
                if os.environ.get("KERNEL_SKIP_AR"):
                    for k in range(DK):
                        nc.vector.tensor_add(x_f[k][:], x_f[k][:], ya[k][:])
                    return
                arin = pdram.tile([D, SP], BF16, name="arin", tag=f"ari{tag}")
                arout = pdram.tile([D, SP], BF16, name="arout", tag=f"aro{tag}")
                for m in range(DK):
                    nc.sync.dma_start(arin[m * 128:(m + 1) * 128, :], ya[m][:])
                nc.gpsimd.collective_compute(
                    "AllReduce", ALU.add, replica_groups=RG,
                    ins=[arin.opt()], outs=[arout.opt()],
                )
                for k in range(DK):
                    ab = pwork1.tile([128, SP], BF16, name="ab", tag="ya0")
                    nc.sync.dma_start(ab[:], arout[k * 128:(k + 1) * 128, :])
                    nc.vector.tensor_add(x_f[k][:], x_f[k][:], ab[:])

            with tc.tile_pool(name="pwork1", bufs=1) as pwork1, \
                 tc.tile_pool(name="pwork2", bufs=2) as pwork2, \
                 tc.tile_pool(name="pstat", bufs=1) as pstat:
                L_EFF = int(os.environ.get("KERNEL_LAYERS", str(L)))
                SKIP_ATTN = bool(os.environ.get("KERNEL_SKIP_ATTN"))
                SKIP_LN = bool(os.environ.get("KERNEL_SKIP_LN"))
                SKIP_GEMM = bool(os.environ.get("KERNEL_SKIP_GEMM"))
                def layer_norm_fake(lidx, gbt, out_tag):
                    ht = [pwork1.tile([128, SP], BF16, name="ht", tag=f"ht{k}") for k in range(DK)]
                    for k in range(DK):
                        nc.vector.tensor_copy(ht[k][:], x_f[k][:])
                    return ht
                if SKIP_LN:
                    layer_norm = layer_norm_fake
                for r, l in [(r, l) for r in range(repeat) for l in range(L_EFF)]:
                    rl = f"{r}_{l}" if repeat > 1 else str(l)
                    kvp = (r * L + l) % 2
                    kT = {l: kT_sets[kvp]}
                    vn = {l: vn_sets[kvp]}
                    if l == 0:
                        for k in range(DK):
                            nc.gpsimd.dma_start(x_f[k][:], xt_d[k * 128:(k + 1) * 128, :])
                    # ===== ln1 =====
                    ht = layer_norm(l, (g1t, b1t), f"h1{rl}")

                    # ===== qkv GEMM: qkT = [q(256) | k(256)] x SP =====
                    qT = [pwork1.tile([128, SP], BF16, name="qT", tag=f"wk{i}") for i in range(2)]
                    with tc.tile_pool(name=f"ps_qkv{rl}", bufs=3, space="PSUM") as psq:
                        for m in range(4):
                            dst = qT[m] if m < 2 else kT[l][m - 2]
                            bc = l * 4 + m
                            for (c0, cn) in NBLK:
                                acc = psq.tile([128, 512], F32, name="acc", tag="acc")
                                for k in range(DK):
                                    nc.tensor.matmul(acc[:, :cn], lhsT=w_qk[l][k][:, m * 128:(m + 1) * 128],
                                                     rhs=ht[k][:, c0:c0 + cn],
                                                     start=(k == 0), stop=(k == DK - 1))
                                nc.scalar.activation(dst[:, c0:c0 + cn], acc[:, :cn], AF.Identity,
                                                     bias=qkb_sb[:, bc:bc + 1])
                        # ===== V token-major: vn[st] = [<=128 tokens, 4x(48+ones)] =====
                        for st in range(NT):
                            rn = min(128, SP - st * 128)
                            vacc = psq.tile([128, SEG], F32, name="vacc", tag="vacc")
                            for k in range(DK):
                                nc.tensor.matmul(vacc[:rn, :], lhsT=ht[k][:, st * 128:st * 128 + rn],
                                                 rhs=w_v[l][k][:], start=(k == 0), stop=False)
                            nc.tensor.matmul(vacc[:rn, :], lhsT=ones_mb[:, :rn], rhs=bias_v[l][:],
                                             start=False, stop=True)
                            nc.vector.tensor_copy(vn[l][:rn, st * SEG:(st + 1) * SEG], vacc[:rn, :])
                    for h in range(NHC):
                        nc.sync.dma_start(kout[l, h * HD:(h + 1) * HD, :],
                                          kT[l][h // 2][64 * (h % 2):64 * (h % 2) + HD, :SD])
                    for st in range(NT):
                        rn = min(128, SD - st * 128)
                        nc.sync.dma_start(vout[l, st * 128:st * 128 + rn, :],
                                          vn[l][:rn, st * SEG:(st + 1) * SEG])

                    # ===== attention -> ctxT [256, SP] bf16 =====
                    ctxT = [pwork1.tile([128, SP], BF16, name="ctxT", tag=f"wk{i + 2}") for i in range(2)]
                    for i in range(2):
                        nc.vector.memset(ctxT[i][:], 0.0)
                    QBLK = [(0, 512, 3), (512, 512, 7), (1024, 32, 8)]  # (col0, width, jmax)
                    with tc.tile_pool(name=f"ps_att{rl}", bufs=2, space="PSUM") as psa:
                        if SKIP_ATTN:
                            QBLK = []
                        for (c0, cn, jmax) in QBLK:
                            t0 = c0 // 128
                            for h in range(NHC):
                                htile, hrow = h // 2, 64 * (h % 2)
                                cp = psa.tile([HD + 1, 512], F32, name="cp", tag="cp")  # row0=denom, rows1..48=ctx
                                for j in range(jmax + 1):
                                    rj = min(128, SP - j * 128)
                                    sp = psa.tile([128, 512], F32, name="sp", tag="sp")
                                    nc.tensor.matmul(sp[:rj, :cn],
                                                     lhsT=kT[l][htile][hrow:hrow + 64, j * 128:j * 128 + rj],
                                                     rhs=qT[htile][hrow:hrow + 64, c0:c0 + cn],
                                                     start=True, stop=True)
                                    if j >= t0:
                                        msk = m_last if c0 == 1024 else m_qd[j - t0]
                                        nc.vector.tensor_add(sp[:rj, :cn], sp[:rj, :cn], msk[:rj, :cn])
                                    et = pwork2.tile([128, 512], BF16, name="et", tag="w512")
                                    nc.scalar.activation(et[:rj, :cn], sp[:rj, :cn], AF.Exp)
                                    nc.tensor.matmul(cp[:, :cn],
                                                     lhsT=vn[l][:rj, st_h(j, h):st_h(j, h) + HD + 1],
                                                     rhs=et[:rj, :cn], start=(j == 0), stop=(j == jmax))
                                dr = pwork2.tile([1, 512], BF16, name="dr", tag="dr")
                                with nc.allow_low_precision(reason="softmax denom bf16 broadcast rhs"):
                                    nc.vector.reciprocal(dr[:, :cn], cp[0:1, :cn])
                                rb = psa.tile([HD + 1, 512], F32, name="rb", tag="rb")
                                nc.tensor.matmul(rb[:, :cn], lhsT=ones_mb[:, :HD + 1], rhs=dr[:, :cn], start=True, stop=True)
                                rbs = pwork2.tile([HD + 1, 512], F32, name="rbs", tag="rbs")
                                nc.scalar.copy(rbs[:, :cn], rb[:, :cn])
                                # row0 becomes exactly denom/denom = 1; wo row0 carries out_b
                                nc.vector.tensor_mul(
                                    ctxT[htile][hrow:hrow + HD + 1, c0:c0 + cn], cp[:, :cn], rbs[:, :cn])

                    # ===== out-proj (row-parallel) + AR + residual =====
                    ya = [pwork1.tile([128, SP], BF16, name="ya", tag=f"ya{m % 3}") for m in range(DK)]
                    with tc.tile_pool(name=f"ps_out{rl}", bufs=3, space="PSUM") as pso:
                        for m in range(DK):
                            for (c0, cn) in NBLK:
                                acc = pso.tile([128, 512], F32, name="acc", tag="acc")
                                for k in range(2):
                                    nc.tensor.matmul(acc[:, :cn], lhsT=w_o[l][k][:, m * 128:(m + 1) * 128],
                                                     rhs=ctxT[k][:, c0:c0 + cn], start=(k == 0), stop=(k == 1))
                                nc.scalar.copy(ya[m][:, c0:c0 + cn], acc[:, :cn])
                    allreduce_add(l, ya, f"o{rl}")

                    # ===== ln2 + ff1 + relu =====
                    h2 = layer_norm(l, (g2t, b2t), f"h2{rl}")
                    fT = [pwork1.tile([128, SP], BF16, name="fT", tag=f"wk{m}") for m in range(DK)]
                    with tc.tile_pool(name=f"ps_ff1{rl}", bufs=3, space="PSUM") as psf:
                        for m in range(DK):
                            for (c0, cn) in NBLK:
                                acc = psf.tile([128, 512], F32, name="acc", tag="acc")
                                for k in range(DK):
                                    nc.tensor.matmul(acc[:, :cn], lhsT=w_f1[l][k][:, m * 128:(m + 1) * 128],
                                                     rhs=h2[k][:, c0:c0 + cn], start=(k == 0), stop=(k == DK - 1))
                                bc = l * DK + m
                                nc.scalar.activation(fT[m][:, c0:c0 + cn], acc[:, :cn], AF.Relu,
                                                     bias=f1b_sb[:, bc:bc + 1])
                    # ===== ff2 (row-parallel) + AR + residual =====
                    ya2 = [pwork1.tile([128, SP], BF16, name="ya2", tag=f"ya{(m + 3) % 3}") for m in range(DK)]
                    with tc.tile_pool(name=f"ps_ff2{rl}", bufs=3, space="PSUM") as psg:
                        for m in range(DK):
                            for (c0, cn) in NBLK:
                                acc = psg.tile([128, 512], F32, name="acc", tag="acc")
                                for k in range(DK):
                                    nc.tensor.matmul(acc[:, :cn], lhsT=w_f2[l][k][:, m * 128:(m + 1) * 128],
                                                     rhs=fT[k][:, c0:c0 + cn], start=(k == 0), stop=(k == DK - 1))
                                bc = l * DK + m
                                nc.scalar.activation(ya2[m][:, c0:c0 + cn], acc[:, :cn], AF.Identity,
                                                     bias=f2b_sb[:, bc:bc + 1])
                    allreduce_add(l, ya2, f"f{rl}")

                # final: export last x tile columns (token 1040 lives at col 1024+16)
                for k in range(DK):
                    nc.gpsimd.dma_start(xlast[k * 128:(k + 1) * 128, :], x_f[k][:, 1024:1056])

    nc.compile()
    _cache[key] = nc
    return nc


# ---------------------------------------------------------------------------
# warm runner: build the sharded executable once, keep inputs device-resident,
# execute many times. run_bass_kernel_spmd builds a fresh jax.jit closure per
# call, so every launch would re-link/load the NEFF on top of executing it.
# ---------------------------------------------------------------------------

class WarmRunner:
    def __init__(self, nc, in_maps, n_cores):
        import jax
        import jax.numpy as jnp
        from jax.sharding import Mesh, PartitionSpec, NamedSharding
        from jax.experimental.shard_map import shard_map
        from concourse.bass2jax import (
            _bass_exec_p, partition_id_tensor, install_neuronx_cc_hook,
        )
        self._jax, self._np = jax, np
        install_neuronx_cc_hook()
        partition_name = (
            nc.partition_id_tensor.name if nc.partition_id_tensor else None
        )
        in_names, out_names, out_avals, zero_shapes = [], [], [], []
        for alloc in nc.m.functions[0].allocations:
            if not isinstance(alloc, mybir.MemoryLocationSet):
                continue
            name = alloc.memorylocations[0].name
            if alloc.kind == "ExternalInput":
                if name != partition_name:
                    in_names.append(name)
            elif alloc.kind == "ExternalOutput":
                shape = tuple(alloc.tensor_shape)
                dtype = mybir.dt.np(alloc.dtype)
                out_names.append(name)
                out_avals.append(jax.core.ShapedArray(shape, dtype))
                zero_shapes.append((shape, dtype))
        n_params = len(in_names)
        n_outs = len(out_avals)
        all_in_names = list(in_names) + list(out_names)
        if partition_name is not None:
            all_in_names.append(partition_name)
        donate = tuple(range(n_params, n_params + n_outs))

        def _body(*args):
            operands = list(args)
            if partition_name is not None:
                operands.append(partition_id_tensor())
            outs = _bass_exec_p.bind(
                *operands,
                out_avals=tuple(out_avals),
                in_names=tuple(all_in_names),
                out_names=tuple(out_names),
                lowering_input_output_aliases=(),
                sim_require_finite=True,
                sim_require_nnan=True,
                nc=nc,
            )
            return tuple(outs)

        devices = jax.devices()[:n_cores]
        mesh = Mesh(np.asarray(devices), ("core",))
        in_specs = (PartitionSpec("core"),) * (n_params + n_outs)
        out_specs = (PartitionSpec("core"),) * n_outs
        self._sharded = jax.jit(
            shard_map(_body, mesh=mesh, in_specs=in_specs,
                      out_specs=out_specs, check_rep=False),
            donate_argnums=donate,
            keep_unused=True,
        )
        sh = NamedSharding(mesh, PartitionSpec("core"))
        concat_in = [
            np.concatenate([np.asarray(m[nm]) for m in in_maps], axis=0)
            for nm in in_names
        ]
        self._dev_in = [jax.device_put(a, sh) for a in concat_in]
        jax.block_until_ready(self._dev_in)

        def _mk_zeros():
            return tuple(
                jnp.zeros((n_cores * s[0], *s[1:]), d) for (s, d) in zero_shapes
            )

        self._mk_zeros = jax.jit(_mk_zeros, out_shardings=(sh,) * n_outs)
        self._out_names = out_names
        self._out_avals = out_avals
        self._n_cores = n_cores

    def run_raw(self):
        zeros = self._mk_zeros()
        self._jax.block_until_ready(zeros)
        outs = self._sharded(*self._dev_in, *zeros)
        self._jax.block_until_ready(outs)
        return outs

    def time_ns(self, reps=8, warmup=2):
        import time as _t
        for _ in range(warmup):
            self.run_raw()
        ts = []
        for _ in range(reps):
            zeros = self._mk_zeros()
            self._jax.block_until_ready(zeros)
            t0 = _t.perf_counter_ns()
            outs = self._sharded(*self._dev_in, *zeros)
            self._jax.block_until_ready(outs)
            ts.append(_t.perf_counter_ns() - t0)
        return ts

    def results(self):
        outs = self.run_raw()
        res = []
        for c in range(self._n_cores):
            m = {}
            for i, nm in enumerate(self._out_names):
                shp = self._out_avals[i].shape
                m[nm] = np.asarray(outs[i]).reshape(self._n_cores, *shp)[c]
            res.append(m)
        return res


# ---------------------------------------------------------------------------
# host side
# ---------------------------------------------------------------------------

def _ln_np(x, g, b, eps=1e-5):
    m = x.mean(-1, keepdims=True)
    v = ((x - m) ** 2).mean(-1, keepdims=True)
    return (x - m) / np.sqrt(v + eps) * g + b


def _gelu(z):
    from scipy.special import erf
    return 0.5 * z * (1 + erf(z / np.sqrt(2)))


def _prep_seq0(i):
    img = i['feats'].transpose(0, 2, 1) + i['img_pos_enc']
    it = i['intent_embeds'][np.clip(i['intent'].astype(np.int64) - 1, 0, 2)][:, None, :]
    past = i['past'] @ i['past_W'] + i['past_b'] + i['time_embeds'][:N_PAST]
    return np.concatenate([img, it, past], 1).astype(np.float32)  # [B, 1041, D]


def _make_in_maps(i):
    seq0 = _prep_seq0(i)
    sc = 1.0 / np.sqrt(HD)
    masks = {}
    # scoresT layout: ROW = key, COL = query -> allowed iff key <= query.
    # m_qd[o]: mask for key-tile at diagonal offset o within a 512-wide q block
    # (q tiles t: t<o fully masked, t==o triangular, t>o unmasked).
    r = np.arange(128)
    tri = r[:, None] <= r[None, :]
    md = np.zeros((4, 128, 512), np.float32)
    for o in range(4):
        for t in range(4):
            blk = md[o][:, t * 128:(t + 1) * 128]
            if t < o:
                blk[:] = NEG
            elif t == o:
                blk[:] = np.where(tri, 0.0, NEG)
    mdq = md.astype(NPBF16)
    # last 32-wide q block (tokens 1024..1055): key rows are tile 8 (tokens
    # 1024..1055). real keys: <=1040 (i<=16). pad queries (qq>=17) attend all
    # real keys so their softmax stays finite.
    r32 = np.arange(32)
    ml = np.where((r32[:, None] <= 16) & ((r32[:, None] <= r32[None, :]) | (r32[None, :] >= 17)),
                  0.0, NEG).astype(NPBF16)
    in_maps = []
    for core in range(NC):
        g, c = divmod(core, TP)
        heads = range(NHC * c, NHC * (c + 1))
        xt = np.zeros((D, SP), np.float32)
        xt[:, :S0] = seq0[g].T
        # --- weight slices, head-padded to HDP ---
        wqk = np.zeros((L, D, 2 * NHC * HDP), np.float32)
        qkbias = np.zeros((L, 1, 2 * NHC * HDP), np.float32)  # transposed at pack time
        wvv = np.zeros((L, D, SEG), np.float32)
        vbias = np.zeros((L, 1, SEG), np.float32)
        vbias[:, :, ::HD + 1] = 1.0  # leading ones-column per head -> softmax denom on partition 0
        wob = np.zeros((L, NHC * HDP, D), np.float32)
        for l in range(L):
            Wq = i['qkv_W'][l][:, :D]
            Wk = i['qkv_W'][l][:, D:2 * D]
            Wv = i['qkv_W'][l][:, 2 * D:]
            bq = i['qkv_b'][l][:D]
            bk = i['qkv_b'][l][D:2 * D]
            bv = i['qkv_b'][l][2 * D:]
            Wo = i['out_W'][l]
            for hi, h in enumerate(heads):
                cs = slice(h * HD, (h + 1) * HD)
                wqk[l, :, hi * HDP:hi * HDP + HD] = Wq[:, cs] * sc
                qkbias[l, 0, hi * HDP:hi * HDP + HD] = bq[cs] * sc
                wqk[l, :, NHC * HDP + hi * HDP:NHC * HDP + hi * HDP + HD] = Wk[:, cs]
                qkbias[l, 0, NHC * HDP + hi * HDP:NHC * HDP + hi * HDP + HD] = bk[cs]
                wvv[l, :, hi * (HD + 1) + 1:(hi + 1) * (HD + 1)] = Wv[:, cs]
                vbias[l, 0, hi * (HD + 1) + 1:(hi + 1) * (HD + 1)] = bv[cs]
                wob[l, hi * HDP + 1:hi * HDP + 1 + HD, :] = Wo[cs, :]
        if c == 0:
            wob[:, 0, :] = i['out_b']  # rides the ones-row of head 0 (once per TP group)
        f1s = slice(c * FFC, (c + 1) * FFC)
        bcm = os.environ.get("KERNEL_BCAST", "none")
        zw = (core >= TP) and bcm in ("all", "w")
        zx = (c != 0) and bcm in ("all", "xt")
        z16 = lambda a: np.zeros_like(a) if zw else a
        im = {
            "xt": np.zeros((D, SP), NPBF16) if zx else xt.astype(NPBF16),
            "wqk": z16(wqk.astype(NPBF16)),
            "wv": z16(wvv.astype(NPBF16)),
            "wo": z16(wob.astype(NPBF16)),
            "wf1": z16(np.ascontiguousarray(i['ff1_W'][:, :, f1s]).astype(NPBF16)),
            "wf2": z16(np.ascontiguousarray(i['ff2_W'][:, f1s, :]).astype(NPBF16)),
            "qkb": qkbias.transpose(0, 2, 1).astype(np.float32),
            "vb": vbias.astype(NPBF16),
            "f1b": np.ascontiguousarray(i['ff1_b'][:, f1s, None]).astype(np.float32),
            "f2b": (i['ff2_b'][:, :, None] if c == 0 else np.zeros((L, D, 1))).astype(np.float32),
            "gb1": np.ascontiguousarray(np.stack([i['ln1_g'], i['ln1_b']], 1)).astype(NPBF16),
            "gb2": np.ascontiguousarray(np.stack([i['ln2_g'], i['ln2_b']], 1)).astype(NPBF16),
            "mqd": mdq,
            "mlast": ml,
        }
        in_maps.append(im)
    return in_maps


def _host_decode(i, K, V, x_final):
    """20 AR steps with device-filled KV caches. K,V: [L,B,SMAX,H,HD] fp32."""
    preds = []
    h_out = x_final[:, None, :]                      # [B,1,D]
    for t in range(T_DEC):
        if t > 0:
            pos = S0 + t - 1                         # index of the new token
            x = nxt
            for l in range(L):
                h = _ln_np(x, i['ln1_g'][l], i['ln1_b'][l])
                qkv = h @ i['qkv_W'][l] + i['qkv_b'][l]
                q, k, v = np.split(qkv, 3, -1)
                K[l][:, pos] = k.reshape(B, H, HD)
                V[l][:, pos] = v.reshape(B, H, HD)
                q = q.reshape(B, 1, H, HD)
                kk = K[l][:, :pos + 1]
                vv = V[l][:, :pos + 1]
                s = np.einsum('bqhd,bkhd->bhqk', q, kk) / np.sqrt(HD)
                e = np.exp(s - s.max(-1, keepdims=True))
                a = e / e.sum(-1, keepdims=True)
                ctx = np.einsum('bhqk,bkhd->bqhd', a, vv).reshape(B, 1, H * HD)
                x = x + ctx @ i['out_W'][l] + i['out_b'][l]
                h2 = _ln_np(x, i['ln2_g'][l], i['ln2_b'][l])
                x = x + np.maximum(h2 @ i['ff1_W'][l] + i['ff1_b'][l], 0) @ i['ff2_W'][l] + i['ff2_b'][l]
            h_out = x
        p = _gelu(h_out @ i['dec1_W'] + i['dec1_b']) @ i['dec2_W'] + i['dec2_b']
        preds.append(p)
        nxt = (p @ i['pos_W'] + i['pos_b'] + i['time_embeds'][N_PAST + t]).astype(np.float32)
    return np.concatenate(preds, 1).astype(np.float32)


def kernel(**inputs):
    import time as _time
    t0 = _time.perf_counter()
    i = {k: np.asarray(v) for k, v in inputs.items()}
    if os.environ.get("KERNEL_HOST_ONLY"):
        return _host_forward_ref(i)
    nc = _build_prefill_neff()
    t1 = _time.perf_counter()
    in_maps = _make_in_maps(i)
    t2 = _time.perf_counter()
    if os.environ.get("KERNEL_NO_WARM"):
        res = run_bass_kernel_spmd(nc, in_maps, core_ids=list(range(NC)), trace=False).results
    else:
        runner = WarmRunner(nc, in_maps, NC)
        _cache["runner"] = runner
        res = runner.results()
    t3 = _time.perf_counter()

    SMAX = S0 + T_DEC
    K = [np.zeros((B, SMAX, H, HD), np.float32) for _ in range(L)]
    V = [np.zeros((B, SMAX, H, HD), np.float32) for _ in range(L)]
    x_final = np.zeros((B, D), np.float32)
    for core in range(NC):
        g, c = divmod(core, TP)
        ko = res[core]["kout"].astype(np.float32)   # [L, 192, 1056]
        vo = res[core]["vout"].astype(np.float32)   # [L, 1056, 196]
        for l in range(L):
            for hi in range(NHC):
                h = NHC * c + hi
                K[l][g, :S0, h, :] = ko[l, hi * HD:(hi + 1) * HD, :S0].T
                V[l][g, :S0, h, :] = vo[l, :S0, hi * (HD + 1) + 1:(hi + 1) * (HD + 1)]
        if c == 0:
            x_final[g] = res[core]["xlast"].astype(np.float32)[:, 16]
    t4 = _time.perf_counter()
    out = _host_decode(i, K, V, x_final)
    t5 = _time.perf_counter()
    if os.environ.get("KERNEL_TIMING"):
        print(f"[kernel] compile={t1 - t0:.2f}s prep={t2 - t1:.2f}s launch={t3 - t2:.2f}s "
              f"extract={t4 - t3:.2f}s decode={t5 - t4:.2f}s", flush=True)
    return out


def _host_forward_ref(i):
    """Pure-host fallback (debug): full KV-cache forward in numpy fp32."""
    seq0 = _prep_seq0(i)
    SMAX = S0 + T_DEC
    K = [np.zeros((B, SMAX, H, HD), np.float32) for _ in range(L)]
    V = [np.zeros((B, SMAX, H, HD), np.float32) for _ in range(L)]
    x = seq0
    for l in range(L):
        h = _ln_np(x, i['ln1_g'][l], i['ln1_b'][l])
        qkv = h @ i['qkv_W'][l] + i['qkv_b'][l]
        q, k, v = np.split(qkv, 3, -1)
        T = x.shape[1]
        K[l][:, :T] = k.reshape(B, T, H, HD)
        V[l][:, :T] = v.reshape(B, T, H, HD)
        q = q.reshape(B, T, H, HD)
        s = np.einsum('bqhd,bkhd->bhqk', q, K[l][:, :T]) / np.sqrt(HD)
        mask = np.tril(np.ones((T, T), dtype=bool))
        s = np.where(mask[None, None], s, -np.inf)
        e = np.exp(s - s.max(-1, keepdims=True))
        a = e / e.sum(-1, keepdims=True)
        ctx = np.einsum('bhqk,bkhd->bqhd', a, V[l][:, :T]).reshape(B, T, H * HD)
        x = x + ctx @ i['out_W'][l] + i['out_b'][l]
        h2 = _ln_np(x, i['ln2_g'][l], i['ln2_b'][l])
        x = x + np.maximum(h2 @ i['ff1_W'][l] + i['ff1_b'][l], 0) @ i['ff2_W'][l] + i['ff2_b'][l]
    return _host_decode(i, K, V, x[:, -1, :])

